# revision 1
# baseline (speedup 1.0000x reference)
"""Trainium2 Bass kernel for EnhancedMultiHeadSelfAttention (dense transformer block).

Sharding: sequence-parallel over 8 cores. Each core owns L/8 = 256 query rows.
LN1 + K/V projection for all 2048 tokens are replicated on every core (cheaper
than on-chip AllReduce at this size); scores/softmax/attn@V/out-proj/LN2/FFN are
computed only for the core's own 256 rows. No collectives.

Layout: activations are kept feature-major ("transposed", [feature, token]) so
every linear layer is matmul(out=[cols, tok], lhsT=W[k,cols], rhs=actT[k,tok])
with natural weight layout and no on-device transposes. All matmuls run as
float32r (full fp32 data, bf16-rate PE throughput for free dim >= 256).

Math notes:
 - clip(scores,-10,10) never binds: |cos|*0.125 + bias in [-0.125, 0.225].
 - softmax needs no max-subtraction for the same reason.
 - the query-side half of the lcc bias is a per-query constant factor in
   exp-space and cancels in softmax normalization; only the key-side half is
   applied (as per-partition ACT bias in the exp).
 - softmax denominators come from an appended ones-column in V.
 - LN gains/biases are folded into the following matmul's weights on the host.
"""

import numpy as np

import concourse.bass as bass
import concourse.tile as tile
from concourse import bacc, mybir
from concourse.bass_utils import run_bass_kernel_spmd

F32 = mybir.dt.float32
F32R = mybir.dt.float32r

L = 2048          # sequence length
D = 1024          # model dim
H = 16            # heads
DH = 64           # head dim
FF = 4096         # ffn hidden
P = 128           # partitions
NCORES = 8
LQ = L // NCORES  # 256 own query rows per core
DC = D // P       # 8 d-model chunks
FC = FF // P      # 32 ffn chunks
KC = L // P       # 16 key chunks
NBLK = 4          # token blocks of 512 for the replicated phase
BLK = L // NBLK   # 512

# CoreSim doesn't implement Gelu; test_sim swaps this to Identity and checks
# against a gelu-less reference. Hardware always uses the real (erf) Gelu.
GELU_FUNC = mybir.ActivationFunctionType.Gelu

LN_EPS = 1e-5
NORM_EPS = 1e-12
SCALING = DH ** -0.5
LCC = 0.1


def _mm(nc, out, lhsT, rhs, start, stop):
    assert lhsT.dtype == F32R and rhs.dtype == F32R, (lhsT.dtype, rhs.dtype)
    nc.tensor.matmul(out, lhsT, rhs, start=start, stop=stop)


def emit(tc):
    nc = tc.nc

    xt = nc.dram_tensor("xt", [D, L], F32R, kind="ExternalInput").ap()
    xot = nc.dram_tensor("xot", [D, LQ], F32R, kind="ExternalInput").ap()
    wq = nc.dram_tensor("wq", [D, D], F32R, kind="ExternalInput").ap()
    wk = nc.dram_tensor("wk", [D, D], F32R, kind="ExternalInput").ap()
    wv = nc.dram_tensor("wv", [D, D], F32R, kind="ExternalInput").ap()
    wo = nc.dram_tensor("wo", [D, D], F32R, kind="ExternalInput").ap()
    wf1 = nc.dram_tensor("wf1", [D, FF], F32R, kind="ExternalInput").ap()
    wf2 = nc.dram_tensor("wf2", [FF, D], F32R, kind="ExternalInput").ap()
    bq = nc.dram_tensor("bq", [P, DC], F32, kind="ExternalInput").ap()
    bk = nc.dram_tensor("bk", [P, DC], F32, kind="ExternalInput").ap()
    bv = nc.dram_tensor("bv", [D], F32, kind="ExternalInput").ap()
    bo = nc.dram_tensor("bo", [P, DC], F32, kind="ExternalInput").ap()
    bf1 = nc.dram_tensor("bf1", [P, FC], F32, kind="ExternalInput").ap()
    bf2 = nc.dram_tensor("bf2", [P, DC], F32, kind="ExternalInput").ap()
    lcck = nc.dram_tensor("lcck", [P, KC], F32, kind="ExternalInput").ap()
    selr = nc.dram_tensor("selr", [P, P], F32R, kind="ExternalInput").ap()
    selb = nc.dram_tensor("selb", [H, DC * P], F32R, kind="ExternalInput").ap()
    onesc = nc.dram_tensor("onesc", [P, 3], F32R, kind="ExternalInput").ap()
    ones1r = nc.dram_tensor("ones1r", [1, P], F32R, kind="ExternalInput").ap()
    vones = nc.dram_tensor("vones", [P, KC], F32R, kind="ExternalInput").ap()
    out_t = nc.dram_tensor("out_t", [D, LQ], F32, kind="ExternalOutput").ap()

    xt3 = xt.rearrange("(c p) t -> p c t", p=P)        # [128, 8, 2048]
    xot3 = xot.rearrange("(c p) t -> p c t", p=P)      # [128, 8, 256]
    wq3 = wq.rearrange("(c p) n -> p c n", p=P)        # [128, 8, 1024]
    wk3 = wk.rearrange("(c p) n -> p c n", p=P)
    wv3 = wv.rearrange("(c p) n -> p c n", p=P)
    wo3 = wo.rearrange("(c p) n -> p c n", p=P)
    wf13 = wf1.rearrange("(c p) n -> p c n", p=P)      # [128, 8, 4096]
    wf23 = wf2.rearrange("(c p) n -> p c n", p=P)      # [128, 32, 1024]
    out3 = out_t.rearrange("(c p) t -> p c t", p=P)    # [128, 8, 256]

    # ---- persistent small constants -------------------------------------
    singles = tc.alloc_tile_pool(name="singles", bufs=1)
    ones_1x128 = singles.tile([1, P], F32R)  # K=1 broadcast lhsT
    nc.sync.dma_start(ones_1x128, ones1r)
    onesc_sb = singles.tile([P, 3], F32R)
    nc.sync.dma_start(onesc_sb, onesc)
    ones_col = onesc_sb[:, 0:1]              # K=128 -> M=1 reduction lhsT
    # head-norm selectors (host-precomputed):
    # selr_sb[:, m, h] = 1 if head h belongs to chunk m at this partition;
    # selb_sb[h, m*128+p] = transpose, for broadcasting norms back to chunks
    selr_sb = singles.tile([P, DC, H], F32R)
    nc.sync.dma_start(selr_sb, selr.rearrange("p (m h) -> p m h", h=H))
    selb_sb = singles.tile([H, DC, P], F32R)
    nc.sync.dma_start(selb_sb, selb.rearrange("h (m p) -> h m p", p=P))
    vones_sb = singles.tile([P, KC], F32R)
    nc.sync.dma_start(vones_sb, vones)
    bq_sb = singles.tile([P, DC], F32)
    nc.sync.dma_start(bq_sb, bq)
    bk_sb = singles.tile([P, DC], F32)
    nc.sync.dma_start(bk_sb, bk)
    bo_sb = singles.tile([P, DC], F32)
    nc.sync.dma_start(bo_sb, bo)
    bf1_sb = singles.tile([P, FC], F32)
    nc.sync.dma_start(bf1_sb, bf1)
    bf2_sb = singles.tile([P, DC], F32)
    nc.sync.dma_start(bf2_sb, bf2)
    lcc_sb = singles.tile([P, KC], F32)
    nc.sync.dma_start(lcc_sb, lcck)
    bv_sb = singles.tile([P, D], F32)  # b_v broadcast to all partitions
    nc.sync.dma_start(bv_sb, bass.AP(tensor=bv.tensor, offset=0, ap=[[0, P], [1, D]]))
    eps_sb = singles.tile([1, 1], F32)
    nc.vector.memset(eps_sb, LN_EPS)

    def layer_norm_t(ctx_pool, ps_stat, ps_coef, src_tiles, dst, ncols, sq_pool,
                     src3=None, dst3=None, add_eng=None):
        """LayerNorm along feature dim for feature-major tiles.

        src_tiles: list of DC tiles/APs [128, ncols] (feature chunks)
        dst: [128, DC, ncols] output tile
        """
        sums = ps_stat.tile([1, ncols], F32, tag="stat")
        sumsq = ps_stat.tile([1, ncols], F32, tag="stat")
        for c in range(DC):
            xc = src_tiles[c]
            xsq = sq_pool.tile([P, ncols], F32R, tag="xsq")
            nc.scalar.square(xsq, xc)
            _mm(nc, sums, ones_col, xc, c == 0, c == DC - 1)
            _mm(nc, sumsq, ones_col, xsq, c == 0, c == DC - 1)
        # coeffs on one partition: rstd, shift = -mu*rstd
        mu = ctx_pool.tile([1, ncols], F32, tag="mu")
        nc.vector.tensor_scalar_mul(mu, sums, 1.0 / D)
        ex2 = ctx_pool.tile([1, ncols], F32, tag="ex2")
        nc.vector.tensor_scalar_mul(ex2, sumsq, 1.0 / D)
        var = ctx_pool.tile([1, ncols], F32, tag="var")
        nc.vector.tensor_mul(var, mu, mu)
        nc.vector.tensor_sub(var, ex2, var)
        sd = ctx_pool.tile([1, ncols], F32, tag="sd")
        nc.scalar.activation(sd, var, func=mybir.ActivationFunctionType.Sqrt,
                             bias=eps_sb, scale=1.0)
        rstd = ctx_pool.tile([1, ncols], F32R, tag="rstd")
        with nc.allow_low_precision(reason="f32r matmul operand"):
            nc.vector.reciprocal(rstd, sd)
        shift = ctx_pool.tile([1, ncols], F32R, tag="shift")
        nc.vector.tensor_mul(shift, mu, rstd)
        nc.vector.tensor_scalar_mul(shift, shift, -1.0)
        # broadcast to 128 partitions via K=1 matmul
        rstd_bc = ps_coef.tile([P, ncols], F32, tag="coef")
        shift_bc = ps_coef.tile([P, ncols], F32, tag="coef")
        _mm(nc, rstd_bc, ones_1x128, rstd, True, True)
        _mm(nc, shift_bc, ones_1x128, shift, True, True)
        if dst3 is not None:
            # one 3D op per pass; alternate the add between DVE and GpSimd so
            # neither engine serializes the block pipeline. GpSimd cannot read
            # PSUM, so stage the shift coefficients through SBUF for it.
            rb = rstd_bc.unsqueeze(1).to_broadcast(dst3.shape)
            if add_eng is nc.gpsimd:
                shift_sb = ctx_pool.tile([P, ncols], F32, tag="shift_sb",
                                         bufs=2)
                nc.scalar.copy(shift_sb, shift_bc)
                sb = shift_sb.unsqueeze(1).to_broadcast(dst3.shape)
            else:
                sb = shift_bc.unsqueeze(1).to_broadcast(dst3.shape)
            nc.vector.tensor_mul(dst3, src3, rb)
            add_eng.tensor_add(dst3, dst3, sb)
        else:
            for c in range(DC):
                nc.vector.tensor_mul(dst[:, c, :], src_tiles[c], rstd_bc)
                nc.vector.tensor_add(dst[:, c, :], dst[:, c, :], shift_bc)


    # persistent pools, allocated in reverse-release (stack) order
    vdram_pool = tc.alloc_tile_pool(name="vdram", bufs=1, space="DRAM")
    v_dram = vdram_pool.tile([KC, P, H, DH + 1], F32R)
    x2_pool = tc.alloc_tile_pool(name="x2p", bufs=1)
    x2acc = x2_pool.tile([P, DC, LQ], F32)
    x2 = x2_pool.tile([P, DC, LQ], F32R)
    kt_pool = tc.alloc_tile_pool(name="kt", bufs=1)
    k_t = kt_pool.tile([P, DC, L], F32R)  # [col-in-chunk, chunk, token]
    q_pool = tc.alloc_tile_pool(name="q", bufs=1)
    q_t = q_pool.tile([P, DC, LQ], F32R)
    normed_pool = tc.alloc_tile_pool(name="normed", bufs=1)
    normed_full = normed_pool.tile([P, DC, L], F32R)

    # =====================================================================
    # Phase A: LN1 over all tokens -> normed_full (feature-major, in place)
    # =====================================================================
    with (
        tc.tile_pool(name="ln1sq", bufs=2) as sq_pool,
        tc.tile_pool(name="ln1coef", bufs=1) as coef_small,
        tc.tile_pool(name="ps_stat", bufs=4, space="PSUM") as ps_stat,
        tc.tile_pool(name="ps_coef", bufs=2, space="PSUM") as ps_coef,
    ):
        for b in range(NBLK):
            blk = normed_full[:, :, b * BLK:(b + 1) * BLK]
            eng = nc.sync if b % 2 == 0 else nc.gpsimd
            eng.dma_start(blk, xt3[:, :, b * BLK:(b + 1) * BLK])
            layer_norm_t(coef_small, ps_stat, ps_coef,
                         [blk[:, c, :] for c in range(DC)], blk, BLK, sq_pool,
                         src3=blk, dst3=blk,
                         add_eng=nc.gpsimd if b % 2 == 0 else nc.vector)

    # =====================================================================
    # Phase C: own queries: LN1(own) -> q^T -> cosine-normalize * scaling
    # =====================================================================
    with (
        tc.tile_pool(name="qb", bufs=1) as qb_pool,
        tc.tile_pool(name="qsq", bufs=2) as qsq_pool,
        tc.tile_pool(name="qcoef", bufs=1) as qcoef,
        tc.tile_pool(name="wqstream", bufs=2) as wqstream,
    ):
        normed_own = qb_pool.tile([P, DC, LQ], F32R)
        nc.sync.dma_start(normed_own, xot3)
        with (
            tc.tile_pool(name="ps_stat2", bufs=2, space="PSUM") as ps_stat2,
            tc.tile_pool(name="ps_coef2", bufs=2, space="PSUM") as ps_coef2,
        ):
            layer_norm_t(qcoef, ps_stat2, ps_coef2,
                         [normed_own[:, c, :] for c in range(DC)], normed_own, LQ,
                         qsq_pool)
        with (
            tc.tile_pool(name="ps_mm2", bufs=2, space="PSUM") as ps_mm2,
            tc.tile_pool(name="ps_qn", bufs=2, space="PSUM") as ps_qn,
            tc.tile_pool(name="ps_qbc", bufs=2, space="PSUM") as ps_qbc,
        ):
            for m in range(DC):
                wqm = wqstream.tile([P, DC, P], F32R, tag="wq")
                nc.sync.dma_start(wqm, wq3[:, :, m * P:(m + 1) * P])
                ps = ps_mm2.tile([P, LQ], F32, tag="mm")
                for c in range(DC):
                    _mm(nc, ps, wqm[:, c, :], normed_own[:, c, :], c == 0,
                        c == DC - 1)
                nc.vector.tensor_scalar_add(q_t[:, m, :], ps, bq_sb[:, m:m + 1])
            # cosine-normalize q (x scaling folded into reciprocal)
            nsq = ps_qn.tile([H, LQ], F32, tag="qnsq")
            for m in range(DC):
                qsq = qsq_pool.tile([P, LQ], F32R, tag="xsq")
                nc.scalar.square(qsq, q_t[:, m, :])
                _mm(nc, nsq, selr_sb[:, m, :], qsq, m == 0, m == DC - 1)
            sd = qcoef.tile([H, LQ], F32, tag="qsd", bufs=2)
            nc.scalar.activation(sd, nsq,
                                 func=mybir.ActivationFunctionType.Sqrt,
                                 bias=0.0, scale=1.0)
            nc.vector.tensor_scalar_max(sd, sd, NORM_EPS)
            rec = qcoef.tile([H, LQ], F32R, tag="qrec", bufs=2)
            with nc.allow_low_precision(reason="f32r matmul operand"):
                nc.vector.reciprocal(rec, sd)
            nc.vector.tensor_scalar_mul(rec, rec, SCALING)
            for m in range(DC):
                bc = ps_qbc.tile([P, LQ], F32, tag="qbc")
                _mm(nc, bc, selb_sb[:, m, :], rec, True, True)
                nc.vector.tensor_mul(q_t[:, m, :], q_t[:, m, :], bc)

    # =====================================================================
    # Phase B: V (to DRAM scratch) then K^T + cosine-norm, block-pipelined
    # =====================================================================
    with (
        tc.tile_pool(name="wstream", bufs=2) as wstream,
        tc.tile_pool(name="vstage", bufs=3) as vstage,
        tc.tile_pool(name="knorm", bufs=2) as knorm_pool,
        tc.tile_pool(name="ps_mm", bufs=4, space="PSUM") as ps_mm,
        tc.tile_pool(name="ps_nrm", bufs=1, space="PSUM") as ps_nrm,
        tc.tile_pool(name="ps_nbc", bufs=1, space="PSUM") as ps_nbc,
    ):
        # V natural layout, block-major inside each quarter so the first
        # blocks of normed unblock V matmuls early
        QW = 256
        for n in range(4):
            wvn = wstream.tile([P, DC, QW], F32R, tag="wv")
            nc.gpsimd.dma_start(wvn, wv3[:, :, n * QW:(n + 1) * QW])
            for t in range(KC):
                ps = ps_mm.tile([P, QW], F32, tag="mmv", bufs=2)
                for c in range(DC):
                    _mm(nc, ps, normed_full[:, c, t * P:(t + 1) * P],
                        wvn[:, c, :], c == 0, c == DC - 1)
                stag = vstage.tile([P, 4, DH], F32R, tag="vstage")
                nc.vector.tensor_add(
                    stag, ps.rearrange("p (h d) -> p h d", d=DH),
                    bv_sb[:, n * QW:(n + 1) * QW].rearrange("p (h d) -> p h d",
                                                            d=DH))
                nc.gpsimd.dma_start(v_dram[t, :, n * 4:(n + 1) * 4, 0:DH], stag)
        # K block-outer with inline cosine-normalization, so attention's
        # exp work unblocks per block instead of all at the end
        for b in range(NBLK):
            for m in range(DC):
                wkm = wstream.tile([P, DC, P], F32R, tag="wk")
                nc.sync.dma_start(wkm, wk3[:, :, m * P:(m + 1) * P])
                ps = ps_mm.tile([P, BLK], F32, tag="mm")
                for c in range(DC):
                    _mm(nc, ps, wkm[:, c, :],
                        normed_full[:, c, b * BLK:(b + 1) * BLK], c == 0,
                        c == DC - 1)
                nc.vector.tensor_scalar_add(k_t[:, m, b * BLK:(b + 1) * BLK],
                                            ps, bk_sb[:, m:m + 1])
            nsq = ps_nrm.tile([H, BLK], F32, tag="nsq")
            for m in range(DC):
                ksq = knorm_pool.tile([P, BLK], F32R, tag="ksq")
                nc.scalar.square(ksq, k_t[:, m, b * BLK:(b + 1) * BLK])
                _mm(nc, nsq, selr_sb[:, m, :], ksq, m == 0, m == DC - 1)
            sd = knorm_pool.tile([H, BLK], F32, tag="ksd")
            nc.scalar.activation(sd, nsq,
                                 func=mybir.ActivationFunctionType.Sqrt,
                                 bias=0.0, scale=1.0)
            nc.vector.tensor_scalar_max(sd, sd, NORM_EPS)
            rec = knorm_pool.tile([H, BLK], F32R, tag="krec")
            with nc.allow_low_precision(reason="f32r matmul operand"):
                nc.vector.reciprocal(rec, sd)
            for m in range(DC):
                bc = ps_nbc.tile([P, BLK], F32, tag="nbc")
                _mm(nc, bc, selb_sb[:, m, :], rec, True, True)
                nc.vector.tensor_mul(k_t[:, m, b * BLK:(b + 1) * BLK],
                                     k_t[:, m, b * BLK:(b + 1) * BLK], bc)

    normed_pool.release()

    # =====================================================================
    # Phase D: attention per head-pair, with the out-projection folded in
    # (partial products accumulated into x2acc via DVE)
    # =====================================================================
    with (
        tc.tile_pool(name="exp", bufs=2) as exp_pool,
        tc.tile_pool(name="vsb", bufs=2) as vsb_pool,
        tc.tile_pool(name="rsc", bufs=2) as rsc_pool,
        tc.tile_pool(name="apair", bufs=2) as apair_pool,
        tc.tile_pool(name="wostream", bufs=2) as wostream,
        tc.tile_pool(name="ps_sc", bufs=2, space="PSUM") as ps_sc,
        tc.tile_pool(name="ps_acc", bufs=1, space="PSUM") as ps_acc,
        tc.tile_pool(name="ps_rbc", bufs=1, space="PSUM") as ps_rbc,
        tc.tile_pool(name="ps_op", bufs=2, space="PSUM") as ps_op,
    ):
        for m in range(DC):
            vp = vsb_pool.tile([P, KC, 2, DH + 1], F32R, tag="vp")
            for j in range(2):
                nc.gpsimd.dma_start(
                    vp[:, :, j, 0:DH],
                    v_dram[:, :, 2 * m + j, 0:DH].rearrange("k p d -> p k d"))
                nc.gpsimd.dma_start(
                    vp[:, :, j, DH:DH + 1],
                    vones_sb.rearrange("p (h o) -> p h o", o=1))
            eh = exp_pool.tile([P, KC, 2 * LQ], F32R, tag="exp")
            for kc in range(KC):
                # each head's scores go to a separate PSUM bank: fp32r matmul
                # writes at mid-bank free offsets fault on hardware
                ps = ps_sc.tile([P, 2, 2 * LQ], F32, tag="sc")
                for j in range(2):
                    _mm(nc, ps[:, j, 0:LQ],
                        k_t[j * DH:(j + 1) * DH, m, kc * P:(kc + 1) * P],
                        q_t[j * DH:(j + 1) * DH, m, :], True, True)
                nc.scalar.activation(
                    eh[:, kc, :].rearrange("p (j q) -> p j q", j=2),
                    ps[:, :, 0:LQ],
                    func=mybir.ActivationFunctionType.Exp,
                    bias=lcc_sb[:, kc:kc + 1], scale=1.0)
            attn_pair = apair_pool.tile([P, LQ], F32R, tag="apair")
            for j in range(2):
                acc = ps_acc.tile([DH + 1, LQ], F32, tag="acc")
                for kc in range(KC):
                    _mm(nc, acc, vp[:, kc, j, :],
                        eh[:, kc, j * LQ:(j + 1) * LQ], kc == 0, kc == KC - 1)
                recip = rsc_pool.tile([1, LQ], F32R, tag="recip")
                with nc.allow_low_precision(reason="f32r matmul operand"):
                    nc.vector.reciprocal(recip, acc[DH:DH + 1, :])
                rbc = ps_rbc.tile([DH, LQ], F32, tag="rbc")
                _mm(nc, rbc, ones_1x128[:, 0:DH], recip, True, True)
                rbc_sb = rsc_pool.tile([DH, LQ], F32, tag="rbcsb")
                nc.vector.tensor_copy(rbc_sb, rbc)
                nc.vector.tensor_mul(attn_pair[j * DH:(j + 1) * DH, :],
                                     acc[0:DH, :], rbc_sb)
            # out-projection partial for this pair-chunk of attn
            wom = wostream.tile([P, DC, P], F32R, tag="wo")
            nc.sync.dma_start(wom, wo3.rearrange("p c n -> p c n")[
                :, m, :].rearrange("p (o n) -> p o n", n=P))
            for o in range(DC):
                pso = ps_op.tile([P, LQ], F32, tag="op")
                _mm(nc, pso, wom[:, o, :], attn_pair, True, True)
                if m == 0:
                    nc.vector.tensor_copy(x2acc[:, o, :], pso)
                else:
                    nc.vector.tensor_add(x2acc[:, o, :], x2acc[:, o, :], pso)

    q_pool.release()
    kt_pool.release()

    # =====================================================================
    # Phase E: residual -> x2; LN2; FFN (ff2 single-pass, half-packed psum)
    # =====================================================================
    with (
        tc.tile_pool(name="xo2p", bufs=1) as xo2_pool,
        tc.tile_pool(name="ffsq", bufs=2) as ffsq_pool,
        tc.tile_pool(name="ffcoef", bufs=2) as ffcoef,
        tc.tile_pool(name="ht", bufs=1) as ht_pool,
        tc.tile_pool(name="wf1s", bufs=3) as wf1s,
        tc.tile_pool(name="wf2s", bufs=3) as wf2s,
        tc.tile_pool(name="outsb", bufs=2) as outsb_pool,
    ):
        xo2 = xo2_pool.tile([P, DC, LQ], F32R)
        nc.sync.dma_start(xo2, xot3)
        for o in range(DC):
            nc.vector.tensor_scalar_add(x2[:, o, :], x2acc[:, o, :],
                                        bo_sb[:, o:o + 1])
            nc.vector.tensor_add(x2[:, o, :], x2[:, o, :], xo2[:, o, :])
        normed2 = xo2_pool.tile([P, DC, LQ], F32R)
        with (
            tc.tile_pool(name="ps_stat3", bufs=2, space="PSUM") as ps_stat3,
            tc.tile_pool(name="ps_coef3", bufs=2, space="PSUM") as ps_coef3,
        ):
            layer_norm_t(ffcoef, ps_stat3, ps_coef3,
                         [x2[:, c, :] for c in range(DC)], normed2, LQ,
                         ffsq_pool)
        ps_mm3 = tc.alloc_tile_pool(name="ps_mm3", bufs=3, space="PSUM")
        ps_ff2 = tc.alloc_tile_pool(name="ps_ff2", bufs=4, space="PSUM")
        h_t = ht_pool.tile([P, FC, LQ], F32R)
        wf24 = wf23.rearrange("p c (g n) -> p c g n", g=2)  # [128,32,2,512]
        for f in range(FC):
            wf1m = wf1s.tile([P, DC, P], F32R, tag="wf1")
            weng = nc.sync if f % 2 == 0 else nc.gpsimd
            weng.dma_start(wf1m, wf13[:, :, f * P:(f + 1) * P])
            ps = ps_mm3.tile([P, LQ], F32, tag="mm")
            for c in range(DC):
                _mm(nc, ps, wf1m[:, c, :], normed2[:, c, :], c == 0, c == DC - 1)
            nc.scalar.activation(h_t[:, f, :], ps, func=GELU_FUNC,
                                 bias=bf1_sb[:, f:f + 1], scale=1.0)
        # ff2: f-outer accumulation in two 4-output passes; pass 1 pipelines
        # with ff1 chunk by chunk
        for g in range(2):
            accs = [ps_ff2.tile([P, LQ], F32, tag="ff2acc",
                                name=f"ff2acc_{g}_{i}") for i in range(4)]
            for f in range(FC):
                wf2m = wf2s.tile([P, 4, P], F32R, tag="wf2")
                weng2 = nc.gpsimd if f % 2 == 0 else nc.sync
                weng2.dma_start(wf2m, wf24[:, f, g, :].rearrange(
                    "p (i n) -> p i n", n=P))
                for i in range(4):
                    _mm(nc, accs[i], wf2m[:, i, :], h_t[:, f, :],
                        f == 0, f == FC - 1)
            for i in range(4):
                mcol = g * 4 + i
                osb = outsb_pool.tile([P, LQ], F32, tag="osb")
                nc.vector.tensor_scalar_add(osb, accs[i], bf2_sb[:, mcol:mcol + 1])
                nc.vector.tensor_add(osb, osb, x2[:, mcol, :])
                nc.sync.dma_start(out3[:, mcol, :], osb)
        ps_ff2.release()
        ps_mm3.release()

    x2_pool.release()
    vdram_pool.release()
    singles.release()


_CACHED = None


def build():
    global _CACHED
    if _CACHED is None:
        nc = bacc.Bacc("TRN2", target_bir_lowering=False, debug=False)
        with tile.TileContext(nc) as tc:
            emit(tc)
        nc.compile()
        _CACHED = nc
    return _CACHED


def _onesc_matrix():
    o = np.zeros((P, 3), np.float32)
    o[:, 0] = 1.0
    o[0:DH, 1] = 1.0
    o[DH:P, 2] = 1.0
    return o


def _selr_matrix():
    # [P, DC*H]: selr[p, m*16+h] = 1 iff h == 2m + (p >= 64)
    s = np.zeros((P, DC, H), np.float32)
    for m in range(DC):
        s[0:DH, m, 2 * m] = 1.0
        s[DH:P, m, 2 * m + 1] = 1.0
    return np.ascontiguousarray(s.reshape(P, P))


def _selb_matrix():
    # [H, DC*P]: selb[h, m*128+p] = 1 iff h == 2m + (p >= 64)
    s = np.zeros((H, DC, P), np.float32)
    for m in range(DC):
        s[2 * m, m, 0:DH] = 1.0
        s[2 * m + 1, m, DH:P] = 1.0
    return np.ascontiguousarray(s.reshape(H, DC * P))


def prep_inputs(inputs):
    """Host-side preprocessing: transpose x, split/fold weights, bias layouts."""
    f = np.float32
    x = np.asarray(inputs["x"], f)
    lcc = np.asarray(inputs["lcc_values"], f)
    w_qkv = np.asarray(inputs["w_qkv"], f)
    b_qkv = np.asarray(inputs["b_qkv"], f)
    ln1_g = np.asarray(inputs["ln1_g"], f)
    ln1_b = np.asarray(inputs["ln1_b"], f)
    ln2_g = np.asarray(inputs["ln2_g"], f)
    ln2_b = np.asarray(inputs["ln2_b"], f)
    w_ff1 = np.asarray(inputs["w_ff1"], f)
    b_ff1 = np.asarray(inputs["b_ff1"], f)

    def chunked(b):  # [D] -> [128, DC] with chunk c in column c
        return np.ascontiguousarray(b.reshape(-1, P).T)

    xt = np.ascontiguousarray(x.T)
    shared = {
        "xt": xt,
        "wq": np.ascontiguousarray(ln1_g[:, None] * w_qkv[:, 0:D]),
        "wk": np.ascontiguousarray(ln1_g[:, None] * w_qkv[:, D:2 * D]),
        "wv": np.ascontiguousarray(ln1_g[:, None] * w_qkv[:, 2 * D:3 * D]),
        "wo": np.ascontiguousarray(np.asarray(inputs["w_out"], f)),
        "wf1": np.ascontiguousarray(ln2_g[:, None] * w_ff1),
        "wf2": np.ascontiguousarray(np.asarray(inputs["w_ff2"], f)),
        "bq": chunked(b_qkv[0:D] + ln1_b @ w_qkv[:, 0:D]),
        "bk": chunked(b_qkv[D:2 * D] + ln1_b @ w_qkv[:, D:2 * D]),
        "bv": np.ascontiguousarray(b_qkv[2 * D:3 * D] + ln1_b @ w_qkv[:, 2 * D:3 * D]),
        "bo": chunked(np.asarray(inputs["b_out"], f)),
        "bf1": chunked(b_ff1 + ln2_b @ w_ff1),
        "bf2": chunked(np.asarray(inputs["b_ff2"], f)),
        "lcck": np.ascontiguousarray((lcc * (0.5 * LCC)).reshape(KC, P).T),
        "selr": _selr_matrix(),
        "selb": _selb_matrix(),
        "onesc": _onesc_matrix(),
        "ones1r": np.ones((1, P), np.float32),
        "vones": np.ones((P, KC), np.float32),
    }
    in_maps = []
    for c in range(NCORES):
        m = dict(shared)
        m["xot"] = np.ascontiguousarray(xt[:, c * LQ:(c + 1) * LQ])
        in_maps.append(m)
    return in_maps


def kernel(**inputs):
    nc = build()
    in_maps = prep_inputs(inputs)
    res = run_bass_kernel_spmd(nc, in_maps, core_ids=list(range(NCORES)))
    out = np.concatenate([res.results[c]["out_t"] for c in range(NCORES)], axis=1)
    return np.ascontiguousarray(out.T).astype(np.float32)



# revision 16
# speedup vs baseline: 1.2687x; 1.2687x over previous
"""Trainium2 Bass kernel for EnhancedMultiHeadSelfAttention (dense transformer block).

Sharding: sequence-parallel over 8 cores. Each core owns L/8 = 256 query rows.
K/V for all 2048 tokens are computed on every core from fp8 x; everything else
(Q, scores, softmax, attn@V, out-proj, LN2, FFN) is own-rows only. No
collectives.

Key structure (all activations feature-major [feature, token]):
 - LN1 is never applied to x. Projections run on raw fp8 x with an extra
   K=1 DoubleRow "correction row": out += (-colsum(W))*mu_t + b*sigma_t.
   The per-token rstd then cancels in cosine normalization for Q/K; for V,
   both rstd and the (key-side) lcc softmax bias are folded into a per-key
   scale applied during V's PSUM->SBUF copy, with the softmax denominator
   coming from an extra V column holding sigma*rstd*e^lcc = e^lcc terms.
 - K is cosine-normalized in place (fp8) so the exp over scores has a
   CONSTANT scale and no bias: one Activation op covers both heads of a
   pair (512 elements), halving Act-engine overhead on the exp path.
 - Q/K/V/out-proj and attn@V run as fp8e4 DoubleRow matmuls (two 128-row
   K-tiles per instruction at 0.5 cycles/row). FFN runs in bf16 (fp8 would
   exceed the 2e-2 error budget through the 4096-wide contraction).
 - clip(scores,-10,10) never binds and softmax needs no max-subtraction.

Scale conventions (stored value = scale * true value):
  x8 = 16 x     w{q,k,v,o}8 = 64 w    proj PSUM = 1024 * true
  k8 = 4 k~ then (after normalize) 16 k-hat      q8 = 16 q-hat
  v8 = 4 e^b v~ (b = key lcc bias + ln rstd)     mu8 = 256 mu
  sd8 = 16 sigma    den col = 16 e^b sigma       attn8 = 16 attn
"""

import numpy as np
import ml_dtypes

import concourse.bass as bass
import concourse.tile as tile
from concourse import bacc, mybir
from concourse.bass_utils import run_bass_kernel_spmd

F32 = mybir.dt.float32
F32R = mybir.dt.float32r
BF16 = mybir.dt.bfloat16
F8 = mybir.dt.float8e4
NP_F8 = ml_dtypes.float8_e4m3
NP_BF16 = ml_dtypes.bfloat16

L = 2048          # sequence length
D = 1024          # model dim
H = 16            # heads
DH = 64           # head dim
FF = 4096         # ffn hidden
P = 128           # partitions
NCORES = 8
LQ = L // NCORES  # 256 own query rows per core
DC = D // P       # 8 d-model chunks
FC = FF // P      # 32 ffn chunks
KC = L // P       # 16 key chunks
NBLK = 4          # token blocks of 512
BLK = L // NBLK   # 512

# CoreSim doesn't implement Gelu; test_sim swaps this to Identity and checks
# against a gelu-less reference. Hardware always uses the real (erf) Gelu.
GELU_FUNC = mybir.ActivationFunctionType.Gelu

LN_EPS = 1e-5
SCALING = DH ** -0.5
LCC = 0.1
DR = mybir.MatmulPerfMode.DoubleRow

SX = 16.0     # x8 scale
SW = 64.0     # fp8 weight scale
PS = SX * SW  # = 1024, scale of projection PSUM results
SKV = 4.0     # k~/v~ scale
SK8 = 16.0    # normalized k-hat scale
SQ8 = 16.0    # q8 scale (unit vectors * 16)
SMU = 256.0   # mu8 scale
SSD = 16.0    # sd8 (sigma) scale
SA = 16.0     # attn8 scale


def emit(tc):
    nc = tc.nc
    AF = mybir.ActivationFunctionType

    xt = nc.dram_tensor("xt", [D, L], BF16, kind="ExternalInput").ap()
    xot = nc.dram_tensor("xot", [D, LQ], F32, kind="ExternalInput").ap()
    xotb = nc.dram_tensor("xotb", [D, LQ], BF16, kind="ExternalInput").ap()
    wq8 = nc.dram_tensor("wq8", [P, DC, D], F8, kind="ExternalInput").ap()
    wk8 = nc.dram_tensor("wk8", [P, DC, D], F8, kind="ExternalInput").ap()
    wv8 = nc.dram_tensor("wv8", [P, DC, D], F8, kind="ExternalInput").ap()
    wo8 = nc.dram_tensor("wo8", [P, DC, D], F8, kind="ExternalInput").ap()
    corq = nc.dram_tensor("corq", [1, 2, D], F8, kind="ExternalInput").ap()
    cork = nc.dram_tensor("cork", [1, 2, D], F8, kind="ExternalInput").ap()
    corv = nc.dram_tensor("corv", [1, 2, D], F8, kind="ExternalInput").ap()
    wf1h = nc.dram_tensor("wf1h", [P, FC, DC, P], BF16, kind="ExternalInput").ap()
    wf2h = nc.dram_tensor("wf2h", [P, FC, DC, P], BF16, kind="ExternalInput").ap()
    bo = nc.dram_tensor("bo", [P, DC], F32, kind="ExternalInput").ap()
    bf1 = nc.dram_tensor("bf1", [P, FC], F32, kind="ExternalInput").ap()
    bf2 = nc.dram_tensor("bf2", [P, DC], F32, kind="ExternalInput").ap()
    lccel = nc.dram_tensor("lccel", [P, KC], F32, kind="ExternalInput").ap()
    selr = nc.dram_tensor("selr", [P, P], F8, kind="ExternalInput").ap()
    selrb = nc.dram_tensor("selrb", [P, P], BF16, kind="ExternalInput").ap()
    selb = nc.dram_tensor("selb", [H, DC * P], F32R, kind="ExternalInput").ap()
    onesbd = nc.dram_tensor("onesbd", [P, 1], BF16, kind="ExternalInput").ap()
    ones1r = nc.dram_tensor("ones1r", [1, P], F32R, kind="ExternalInput").ap()
    c64r = nc.dram_tensor("c64r", [1, P], F32R, kind="ExternalInput").ap()
    onescl = nc.dram_tensor("onescl", [P, 1], F32R, kind="ExternalInput").ap()
    out_t = nc.dram_tensor("out_t", [D, LQ], F32, kind="ExternalOutput").ap()

    xt3 = xt.rearrange("(c p) t -> p c t", p=P)        # [128, 8, 2048]
    xot3 = xot.rearrange("(c p) t -> p c t", p=P)      # [128, 8, 256]
    out3 = out_t.rearrange("(c p) t -> p c t", p=P)    # [128, 8, 256]

    mm = nc.tensor.matmul

    # ---- persistent small constants -------------------------------------
    singles = tc.alloc_tile_pool(name="singles", bufs=1)
    ones_1x128 = singles.tile([1, P], F32R)  # K=1 broadcast lhsT (value 1)
    nc.scalar.dma_start(ones_1x128, ones1r)
    c64row = singles.tile([1, P], F32R)      # K=1 broadcast lhsT (value 64)
    nc.scalar.dma_start(c64row, c64r)
    ones_col = singles.tile([P, 1], F32R)    # K=128 -> M=1 reduction lhsT
    nc.scalar.dma_start(ones_col, onescl)
    onesb = singles.tile([P, 1], BF16)       # bf16 reduction lhsT
    nc.scalar.dma_start(onesb, onesbd)
    selr8 = singles.tile([P, DC, H], F8)
    nc.scalar.dma_start(selr8, selr.rearrange("p (m h) -> p m h", h=H))
    selrb_sb = singles.tile([P, DC, H], BF16)
    nc.scalar.dma_start(selrb_sb, selrb.rearrange("p (m h) -> p m h", h=H))
    selb_sb = singles.tile([H, DC, P], F32R)
    nc.scalar.dma_start(selb_sb, selb.rearrange("h (m p) -> h m p", p=P))
    corq_sb = singles.tile([1, 2, D], F8)
    nc.scalar.dma_start(corq_sb, corq)
    cork_sb = singles.tile([1, 2, D], F8)
    nc.scalar.dma_start(cork_sb, cork)
    corv_sb = singles.tile([1, 2, D], F8)
    nc.scalar.dma_start(corv_sb, corv)
    bo_sb = singles.tile([P, DC], F32)
    nc.scalar.dma_start(bo_sb, bo)
    bf1_sb = singles.tile([P, FC], F32)
    nc.scalar.dma_start(bf1_sb, bf1)
    bf2_sb = singles.tile([P, DC], F32)
    nc.scalar.dma_start(bf2_sb, bf2)
    lcce_sb = singles.tile([P, KC], F32)
    nc.scalar.dma_start(lcce_sb, lccel)
    eps_sb = singles.tile([P, 1], F32)
    nc.vector.memset(eps_sb, LN_EPS)
    # fp8 full weights (4 x 8KB/partition)
    wq_sb = singles.tile([P, DC, D], F8)
    nc.sync.dma_start(wq_sb, wq8)
    wk_sb = singles.tile([P, DC, D], F8)
    nc.sync.dma_start(wk_sb, wk8)
    wv_sb = singles.tile([P, DC, D], F8)
    nc.gpsimd.dma_start(wv_sb, wv8)
    wo_sb = singles.tile([P, DC, D], F8)
    nc.gpsimd.dma_start(wo_sb, wo8)
    # per-token stat tensors (filled during phases A/B)
    musd8 = singles.tile([1, 2, L], F8)       # [mu8 ; sd8] rows
    vscale = singles.tile([P, KC], F32)       # (SKV/PS) * rstd * e^lcc
    col8 = singles.tile([P, KC], F8)          # SSD * rstd * e^lcc (den col)
    reck = singles.tile([H, L], F32R)         # 4 / |k~| rows
    stat_r = singles.tile([P, NBLK, 2, 4], F32)   # [p, b, (sum,sumsq), kc%4]
    musd_r = singles.tile([P, NBLK, 2, 4], F8)    # repartitioned mu8/sd8

    scr_pool = tc.alloc_tile_pool(name="scr", bufs=1, space="DRAM")
    scr_st = scr_pool.tile([NBLK, 2, BLK], F32)    # stats rows -> repart
    scr_ms = scr_pool.tile([NBLK, 2, BLK], F8)     # mu/sd repart -> rows

    # ---- persistent activation tensors ----------------------------------
    outp = tc.alloc_tile_pool(name="outp", bufs=1)
    x2 = outp.tile([P, DC, LQ], F32R)
    attn8 = outp.tile([P, DC, LQ], F8)
    wf1s = tc.alloc_tile_pool(name="wf1s", bufs=10)
    wf2s = tc.alloc_tile_pool(name="wf2s", bufs=4)
    midp = tc.alloc_tile_pool(name="midp", bufs=1)
    k8 = midp.tile([P, DC, L], F8)
    VW = 80  # 64 values + 1 denominator + 15 pad (dual-fp8 M%16==0)
    v_sb = midp.tile([P, KC, H, VW], F8)
    q8 = midp.tile([P, DC, LQ], F8)
    x8p = tc.alloc_tile_pool(name="x8p", bufs=1)
    x8 = x8p.tile([P, DC, L], F8)

    # =====================================================================
    # Phase A: x -> fp8, per-token stats; Phase C: Q projection + normalize
    # (emitted together so the scheduler can overlap them)
    # =====================================================================
    with (
        tc.tile_pool(name="xblk", bufs=2) as xblk_pool,
        tc.tile_pool(name="xsqp", bufs=2) as xsq_pool,
        tc.tile_pool(name="stm", bufs=2) as stm_pool,
        tc.tile_pool(name="strow", bufs=2) as strow_pool,
        tc.tile_pool(name="qcp", bufs=1) as qc_pool,
        tc.tile_pool(name="qsqp", bufs=2) as qsq_pool,
        tc.tile_pool(name="ps_st", bufs=1, space="PSUM") as ps_st,
        tc.tile_pool(name="ps_q", bufs=2, space="PSUM") as ps_q,
        tc.tile_pool(name="ps_qn", bufs=1, space="PSUM") as ps_qn,
        tc.tile_pool(name="ps_qb", bufs=1, space="PSUM") as ps_qb,
    ):
        for b in range(NBLK):
            sl = slice(b * BLK, (b + 1) * BLK)
            xblk = xblk_pool.tile([P, DC, BLK], BF16, tag="xblk")
            nc.gpsimd.dma_start(xblk, xt3[:, :, sl])
            with nc.allow_low_precision(reason="fp8 pipeline"):
                nc.scalar.activation(x8[:, :, sl], xblk, func=AF.Copy,
                                     bias=0.0, scale=SX)
                xsqb = xsq_pool.tile([P, DC, BLK], BF16, tag="xsq")
                nc.vector.tensor_mul(xsqb, xblk, xblk)
            sums = ps_st.tile([1, BLK], F32, tag="sums")
            sumsq = ps_st.tile([1, BLK], F32, tag="sumsq")
            for c in range(DC):
                mm(sums, onesb, xblk[:, c, :], start=(c == 0),
                   stop=(c == DC - 1))
                mm(sumsq, onesb, xsqb[:, c, :], start=(c == 0),
                   stop=(c == DC - 1))
            # stage stat rows to SBUF, roundtrip via DRAM to [128, ...] layout
            statrow = strow_pool.tile([1, 2, BLK], F32, tag="strow")
            nc.vector.tensor_copy(statrow[:, 0, :], sums)
            nc.vector.tensor_copy(statrow[:, 1, :], sumsq)
            nc.sync.dma_start(scr_st[b:b + 1], statrow)
            nc.sync.dma_start(
                stat_r[:, b, :, :],
                scr_st[b].rearrange("j (q p) -> p j q", p=P))
            # per-token coefficient math in [128, 4] layout
            mu = stm_pool.tile([P, 4], F32, tag="mu")
            nc.vector.tensor_scalar_mul(mu, stat_r[:, b, 0, :], 1.0 / D)
            ex2 = stm_pool.tile([P, 4], F32, tag="ex2")
            nc.vector.tensor_scalar_mul(ex2, stat_r[:, b, 1, :], 1.0 / D)
            var = stm_pool.tile([P, 4], F32, tag="var")
            nc.vector.tensor_mul(var, mu, mu)
            nc.vector.tensor_sub(var, ex2, var)
            sd = stm_pool.tile([P, 4], F32, tag="sd")
            nc.scalar.activation(sd, var, func=AF.Sqrt, bias=eps_sb, scale=1.0)
            rstd = stm_pool.tile([P, 4], F32, tag="rstd")
            with nc.allow_low_precision(reason="coef"):
                nc.vector.reciprocal(rstd, sd)
            kcs = slice(b * 4, (b + 1) * 4)
            relcc = stm_pool.tile([P, 4], F32, tag="relcc")
            nc.vector.tensor_mul(relcc, rstd, lcce_sb[:, kcs])
            nc.vector.tensor_scalar_mul(vscale[:, kcs], relcc, SKV / PS)
            with nc.allow_low_precision(reason="fp8 pipeline"):
                nc.vector.tensor_scalar_mul(col8[:, kcs], relcc, SSD)
                nc.vector.tensor_scalar_mul(musd_r[:, b, 0, :], mu, SMU)
                nc.vector.tensor_scalar_mul(musd_r[:, b, 1, :], sd, SSD)
            nc.sync.dma_start(
                scr_ms[b].rearrange("j (q p) -> p j q", p=P),
                musd_r[:, b, :, :])
            for j in range(2):
                nc.sync.dma_start(
                    musd8[:, j, sl],
                    scr_ms[b, j].rearrange("(o t) -> o t", o=1))

        # ---- Phase C: Q (own tokens; stats recomputed from xot since the
        # shared program can't address its own slice of musd8) --------------
        xo_blk = qc_pool.tile([P, DC, LQ], BF16, name="xo_blk")
        nc.sync.dma_start(xo_blk, xotb.rearrange("(c p) t -> p c t", p=P))
        x8own = qc_pool.tile([P, DC, LQ], F8, name="x8own")
        with nc.allow_low_precision(reason="fp8 pipeline"):
            nc.scalar.activation(x8own, xo_blk, func=AF.Copy, bias=0.0,
                                 scale=SX)
        ps_os = ps_qn.tile([1, LQ], F32, tag="osum")
        for c in range(DC):
            mm(ps_os, onesb, xo_blk[:, c, :], start=(c == 0),
               stop=(c == DC - 1))
        osr = qc_pool.tile([1, 2, LQ], F32, name="osr")
        nc.vector.tensor_copy(osr[:, 0, :], ps_os)
        xsq_o = qc_pool.tile([P, DC, LQ], BF16, name="xsq_o")
        with nc.allow_low_precision(reason="bf16 pipeline"):
            nc.vector.tensor_mul(xsq_o, xo_blk, xo_blk)
        for c in range(DC):
            mm(ps_os, onesb, xsq_o[:, c, :], start=(c == 0),
               stop=(c == DC - 1))
        nc.vector.tensor_copy(osr[:, 1, :], ps_os)
        mu_o = qc_pool.tile([1, LQ], F32, name="mu_o")
        nc.vector.tensor_scalar_mul(mu_o, osr[:, 0, :], 1.0 / D)
        ex2_o = qc_pool.tile([1, LQ], F32, name="ex2_o")
        nc.vector.tensor_scalar_mul(ex2_o, osr[:, 1, :], 1.0 / D)
        var_o = qc_pool.tile([1, LQ], F32, name="var_o")
        nc.vector.tensor_mul(var_o, mu_o, mu_o)
        nc.vector.tensor_sub(var_o, ex2_o, var_o)
        sd_o = qc_pool.tile([1, LQ], F32, name="sd_o")
        nc.scalar.activation(sd_o, var_o, func=AF.Sqrt, bias=eps_sb[0:1, :],
                             scale=1.0)
        musd_own = qc_pool.tile([1, 2, LQ], F8, name="musd_own")
        with nc.allow_low_precision(reason="fp8 pipeline"):
            nc.vector.tensor_scalar_mul(musd_own[:, 0, :], mu_o, SMU)
            nc.vector.tensor_scalar_mul(musd_own[:, 1, :], sd_o, SSD)
        # Q DoubleRow projections + per-head cosine normalization
        qt_sb = qc_pool.tile([P, DC, LQ], BF16, name="qt_sb")
        nsq_q = ps_qn.tile([H, LQ], F32, tag="qn")
        qsq8 = None
        for m in range(DC):
            ps = ps_q.tile([P, LQ], F32, tag="qps")
            for i in range(4):
                mm(ps, wq_sb[:, 2 * i:2 * i + 2, m * P:(m + 1) * P],
                   x8own[:, 2 * i:2 * i + 2, :], start=(i == 0), stop=False,
                   perf_mode=DR)
            mm(ps, corq_sb[:, :, m * P:(m + 1) * P], musd_own,
               start=False, stop=True, perf_mode=DR)
            with nc.allow_low_precision(reason="bf16 pipeline"):
                nc.vector.tensor_scalar_mul(qt_sb[:, m, :], ps, SKV / PS)
            if m % 2 == 0:
                qsq8 = qsq_pool.tile([P, 2, LQ], F8, tag="qsq")
            nc.scalar.activation(qsq8[:, m % 2, :], ps, func=AF.Square,
                                 bias=0.0, scale=2.0 / PS)
            if m % 2 == 1:
                mm(nsq_q, selr8[:, m - 1:m + 1, :], qsq8,
                   start=(m == 1), stop=(m == DC - 1), perf_mode=DR)
        qs = qc_pool.tile([H, LQ], F32, name="qs")
        nc.scalar.activation(qs, nsq_q, func=AF.Sqrt, bias=0.0, scale=1.0)
        nc.vector.tensor_scalar_max(qs, qs, 1e-8)
        rec = qc_pool.tile([H, LQ], F32R, name="qrec")
        with nc.allow_low_precision(reason="coef"):
            nc.vector.reciprocal(rec, qs)
        nc.vector.tensor_scalar_mul(rec, rec, 2.0 * SQ8 / SKV)
        for m in range(DC):
            bc = ps_qb.tile([P, LQ], F32, tag="qbc")
            mm(bc, selb_sb[:, m, :], rec, start=True, stop=True)
            with nc.allow_low_precision(reason="fp8 pipeline"):
                nc.vector.tensor_mul(q8[:, m, :], qt_sb[:, m, :], bc)

    # sigma/e^lcc column of V (denominator source) + zero pad columns
    with nc.allow_low_precision(reason="fp8 pipeline"):
        nc.vector.memset(v_sb[:, :, :, DH + 1:VW], 0.0)
        nc.vector.tensor_copy(
            v_sb[:, :, :, DH], col8.unsqueeze(2).to_broadcast([P, KC, H]))

    # =====================================================================
    # Phase B: K and V projections (fp8 DR); K cosine-normalized in place
    # =====================================================================
    with (
        tc.tile_pool(name="ksqp", bufs=2) as ksq_pool,
        tc.tile_pool(name="nsqs", bufs=2) as nsqs_pool,
        tc.tile_pool(name="ps_k", bufs=2, space="PSUM") as ps_k,
        tc.tile_pool(name="ps_v", bufs=2, space="PSUM") as ps_v,
        tc.tile_pool(name="ps_n", bufs=2, space="PSUM") as ps_n,
        tc.tile_pool(name="ps_kb", bufs=2, space="PSUM") as ps_kb,
    ):
        for b in range(NBLK):
            sl = slice(b * BLK, (b + 1) * BLK)
            ms = musd8[:, :, sl]
            for m in range(DC):
                ps = ps_k.tile([P, BLK], F32, tag="kps")
                for i in range(4):
                    mm(ps, wk_sb[:, 2 * i:2 * i + 2, m * P:(m + 1) * P],
                       x8[:, 2 * i:2 * i + 2, sl], start=(i == 0), stop=False,
                       perf_mode=DR)
                mm(ps, cork_sb[:, :, m * P:(m + 1) * P], ms,
                   start=False, stop=True, perf_mode=DR)
                with nc.allow_low_precision(reason="fp8 pipeline"):
                    if m % 2 == 0:
                        nc.vector.tensor_scalar_mul(k8[:, m, sl], ps, SKV / PS)
                    else:
                        nc.scalar.activation(k8[:, m, sl], ps, func=AF.Copy,
                                             bias=0.0, scale=SKV / PS)
            # V for this block's 4 token chunks (scaled per key by vscale)
            for t in range(b * 4, (b + 1) * 4):
                tsl = slice(t * P, (t + 1) * P)
                for g in range(2):
                    csl = slice(g * BLK, (g + 1) * BLK)
                    ps = ps_v.tile([P, BLK], F32, tag="vps")
                    for i in range(4):
                        mm(ps, x8[:, 2 * i:2 * i + 2, tsl],
                           wv_sb[:, 2 * i:2 * i + 2, csl],
                           start=(i == 0), stop=False, perf_mode=DR)
                    mm(ps, musd8[:, :, tsl], corv_sb[:, :, csl],
                       start=False, stop=True, perf_mode=DR)
                    ps_h = ps.rearrange("p (h d) -> p h d", d=DH)
                    dst = v_sb[:, t, g * DC:(g + 1) * DC, 0:DH]
                    with nc.allow_low_precision(reason="fp8 pipeline"):
                        if g == 0:
                            nc.vector.tensor_scalar_mul(
                                dst, ps_h, vscale[:, t:t + 1])
                        else:
                            nc.scalar.activation(dst, ps_h, func=AF.Copy,
                                                 bias=0.0,
                                                 scale=vscale[:, t:t + 1])
            # k norms: squares on Pool (bf16), per-head sums, then in-place
            # cosine normalization of k8 (k8 becomes 16 * k-hat)
            nsq = ps_n.tile([H, BLK], F32, tag="nsq")
            for m in range(DC):
                ksqb = ksq_pool.tile([P, BLK], BF16, tag="ksq")
                with nc.allow_low_precision(reason="bf16 pipeline"):
                    nc.gpsimd.tensor_mul(ksqb, k8[:, m, sl], k8[:, m, sl])
                mm(nsq, selrb_sb[:, m, :], ksqb, start=(m == 0),
                   stop=(m == DC - 1))
            nsq_sb = nsqs_pool.tile([H, BLK], F32, tag="nsqs")
            nc.scalar.activation(nsq_sb, nsq, func=AF.Sqrt, bias=0.0,
                                 scale=1.0)
            nc.vector.tensor_scalar_max(nsq_sb, nsq_sb, 1e-8)
            with nc.allow_low_precision(reason="coef"):
                nc.vector.reciprocal(reck[:, sl], nsq_sb)
            nc.vector.tensor_scalar_mul(reck[:, sl], reck[:, sl], SK8)
            for m in range(DC):
                kb = ps_kb.tile([P, BLK], F32, tag="kbc")
                mm(kb, selb_sb[:, m, :], reck[:, sl], start=True, stop=True)
                with nc.allow_low_precision(reason="fp8 pipeline"):
                    nc.vector.tensor_mul(k8[:, m, sl], k8[:, m, sl], kb)

    x8p.release()

    # =====================================================================
    # Phase D: attention per head-pair m: scores -> exp -> attn@V -> attn8
    # =====================================================================
    with (
        tc.tile_pool(name="ehp", bufs=3) as eh_pool,
        tc.tile_pool(name="rcp", bufs=2) as rc_pool,
        tc.tile_pool(name="ps_sc", bufs=2, space="PSUM") as ps_sc,
        tc.tile_pool(name="ps_ac", bufs=2, space="PSUM") as ps_ac,
        tc.tile_pool(name="ps_rb", bufs=2, space="PSUM") as ps_rb,
    ):
        for m in range(DC):
            eh8 = eh_pool.tile([P, KC, 2, LQ], F8, tag="eh")
            for kc in range(KC):
                # both heads' scores in one 2-bank PSUM tile -> one exp op
                ps = ps_sc.tile([P, 2, 2 * LQ], F32, tag="sc")
                for j in range(2):
                    mm(ps[:, j, 0:LQ],
                       k8[j * DH:(j + 1) * DH, m, kc * P:(kc + 1) * P],
                       q8[j * DH:(j + 1) * DH, m, :], start=True, stop=True)
                with nc.allow_low_precision(reason="fp8 pipeline"):
                    nc.scalar.activation(eh8[:, kc, :, :], ps[:, :, 0:LQ],
                                         func=AF.Exp, bias=0.0,
                                         scale=SCALING / (SQ8 * SK8))
            for j in range(2):
                acc = ps_ac.tile([VW, LQ], F32, tag="acc")
                for i in range(KC // 2):
                    mm(acc, v_sb[:, 2 * i:2 * i + 2, 2 * m + j, :],
                       eh8[:, 2 * i:2 * i + 2, j, :],
                       start=(i == 0), stop=(i == KC // 2 - 1), perf_mode=DR)

                recip = rc_pool.tile([1, LQ], F32R, tag="recip")
                with nc.allow_low_precision(reason="coef"):
                    nc.vector.reciprocal(recip, acc[DH:DH + 1, :])
                rbc = ps_rb.tile([DH, LQ], F32, tag="rbc")
                mm(rbc, c64row[:, 0:DH], recip, start=True, stop=True)
                rbc_sb = rc_pool.tile([DH, LQ], F32, tag="rbcsb")
                nc.vector.tensor_copy(rbc_sb, rbc)
                with nc.allow_low_precision(reason="fp8 pipeline"):
                    nc.vector.tensor_mul(
                        attn8[j * DH:(j + 1) * DH, m, :], acc[0:DH, :], rbc_sb)

    # =====================================================================
    # Phase E: out-projection (fp8 DR) + residual -> x2
    # =====================================================================
    with (
        tc.tile_pool(name="xoq", bufs=1) as xoq_pool,
        tc.tile_pool(name="up", bufs=2) as up_pool,
        tc.tile_pool(name="ps_o", bufs=2, space="PSUM") as ps_o,
    ):
        xo2 = xoq_pool.tile([P, DC, LQ], F32, name="xo2")
        nc.sync.dma_start(xo2, xot3)
        for o in range(DC):
            ps = ps_o.tile([P, LQ], F32, tag="ops")
            for i in range(4):
                mm(ps, wo_sb[:, 2 * i:2 * i + 2, o * P:(o + 1) * P],
                   attn8[:, 2 * i:2 * i + 2, :], start=(i == 0), stop=(i == 3),
                   perf_mode=DR)
            upd = up_pool.tile([P, LQ], F32, tag="upd")
            nc.scalar.activation(upd, ps, func=AF.Identity,
                                 bias=bo_sb[:, o:o + 1], scale=1.0 / (SA * SW))
            with nc.allow_low_precision(reason="f32r"):
                nc.vector.tensor_add(x2[:, o, :], upd, xo2[:, o, :])

    midp.release()

    # =====================================================================
    # Phase F: LN2 + FFN (bf16)
    # =====================================================================
    ffp = tc.alloc_tile_pool(name="ffp", bufs=1)
    h_t = ffp.tile([P, FC, LQ], BF16)
    normed2 = ffp.tile([P, DC, LQ], BF16)
    with (
        tc.tile_pool(name="lnc", bufs=1) as lnc_pool,
        tc.tile_pool(name="lsq", bufs=2) as lsq_pool,
        tc.tile_pool(name="ps_l", bufs=1, space="PSUM") as ps_l,
        tc.tile_pool(name="ps_lb", bufs=2, space="PSUM") as ps_lb,
    ):
        sums = ps_l.tile([1, LQ], F32, tag="lsum")
        sumsq = ps_l.tile([1, LQ], F32, tag="lsumsq")
        for c in range(DC):
            xsq = lsq_pool.tile([P, LQ], F32R, tag="lxsq")
            nc.scalar.square(xsq, x2[:, c, :])
            mm(sums, ones_col, x2[:, c, :], start=(c == 0), stop=(c == DC - 1))
            mm(sumsq, ones_col, xsq, start=(c == 0), stop=(c == DC - 1))
        mu = lnc_pool.tile([1, LQ], F32, name="lmu")
        nc.vector.tensor_scalar_mul(mu, sums, 1.0 / D)
        ex2 = lnc_pool.tile([1, LQ], F32, name="lex2")
        nc.vector.tensor_scalar_mul(ex2, sumsq, 1.0 / D)
        var = lnc_pool.tile([1, LQ], F32, name="lvar")
        nc.vector.tensor_mul(var, mu, mu)
        nc.vector.tensor_sub(var, ex2, var)
        sd = lnc_pool.tile([1, LQ], F32, name="lsd")
        nc.scalar.activation(sd, var, func=AF.Sqrt, bias=eps_sb[0:1, :],
                             scale=1.0)
        rstd = lnc_pool.tile([1, LQ], F32R, name="lrstd")
        with nc.allow_low_precision(reason="coef"):
            nc.vector.reciprocal(rstd, sd)
        shift = lnc_pool.tile([1, LQ], F32R, name="lshift")
        nc.vector.tensor_mul(shift, mu, rstd)
        nc.vector.tensor_scalar_mul(shift, shift, -1.0)
        rstd_bc = ps_lb.tile([P, LQ], F32, tag="lcoef")
        shift_bc = ps_lb.tile([P, LQ], F32, tag="lcoef")
        mm(rstd_bc, ones_1x128, rstd, start=True, stop=True)
        mm(shift_bc, ones_1x128, shift, start=True, stop=True)
        shift_sb = lnc_pool.tile([P, LQ], F32, name="lshsb")
        nc.scalar.copy(shift_sb, shift_bc)
        rb = rstd_bc.unsqueeze(1).to_broadcast([P, DC, LQ])
        sb = shift_sb.unsqueeze(1).to_broadcast([P, DC, LQ])
        with nc.allow_low_precision(reason="bf16 pipeline"):
            nc.vector.tensor_mul(normed2, x2, rb)
            nc.gpsimd.tensor_add(normed2, normed2, sb)

    with (
        tc.tile_pool(name="osbp", bufs=2) as osb_pool,
        tc.tile_pool(name="ps_f1", bufs=3, space="PSUM") as ps_f1,
        tc.tile_pool(name="ps_f2", bufs=4, space="PSUM") as ps_f2,
    ):
        for f in range(FC):
            wf1m = wf1s.tile([P, DC, P], BF16, tag="wf1")
            weng = nc.sync if f % 2 == 0 else nc.gpsimd
            weng.dma_start(wf1m, wf1h[:, f, :, :])
            ps = ps_f1.tile([P, LQ], F32, tag="f1")
            for c in range(DC):
                mm(ps, wf1m[:, c, :], normed2[:, c, :], start=(c == 0),
                   stop=(c == DC - 1))
            with nc.allow_low_precision(reason="bf16 pipeline"):
                nc.scalar.activation(h_t[:, f, :], ps, func=GELU_FUNC,
                                     bias=bf1_sb[:, f:f + 1], scale=1.0)
        for g in range(2):
            accs = [ps_f2.tile([P, LQ], F32, tag="f2acc",
                               name=f"f2acc_{g}_{i}") for i in range(4)]
            for f in range(FC):
                wf2m = wf2s.tile([P, 4, P], BF16, tag="wf2")
                weng2 = nc.gpsimd if f % 2 == 0 else nc.sync
                weng2.dma_start(wf2m, wf2h[:, f, g * 4:(g + 1) * 4, :])
                for i in range(4):
                    mm(accs[i], wf2m[:, i, :], h_t[:, f, :],
                       start=(f == 0), stop=(f == FC - 1))
            for i in range(4):
                o = g * 4 + i
                osb = osb_pool.tile([P, LQ], F32, tag="osb")
                nc.scalar.activation(osb, accs[i], func=AF.Identity,
                                     bias=bf2_sb[:, o:o + 1], scale=1.0)
                nc.vector.tensor_add(osb, osb, x2[:, o, :])
                nc.sync.dma_start(out3[:, o, :], osb)

    ffp.release()
    wf2s.release()
    wf1s.release()
    outp.release()
    scr_pool.release()
    singles.release()


_CACHED = None


def build():
    global _CACHED
    if _CACHED is None:
        nc = bacc.Bacc("TRN2", target_bir_lowering=False, debug=False)
        with tile.TileContext(nc) as tc:
            emit(tc)
        nc.compile()
        _CACHED = nc
    return _CACHED


def _selr_matrix():
    # [P, DC*H]: selr[p, m*16+h] = 1 iff h == 2m + (p >= 64)
    s = np.zeros((P, DC, H), np.float32)
    for m in range(DC):
        s[0:DH, m, 2 * m] = 1.0
        s[DH:P, m, 2 * m + 1] = 1.0
    return np.ascontiguousarray(s.reshape(P, P))


def _selb_matrix():
    # [H, DC*P]: selb[h, m*128+p] = 1 iff h == 2m + (p >= 64)
    s = np.zeros((H, DC, P), np.float32)
    for m in range(DC):
        s[2 * m, m, 0:DH] = 1.0
        s[2 * m + 1, m, DH:P] = 1.0
    return np.ascontiguousarray(s.reshape(H, DC * P))


def _chunk_pd(w):
    """[D, N] -> [128, D//128, N] with (p, c, n) = w[c*128+p, n]."""
    Dd, N = w.shape
    return np.ascontiguousarray(w.reshape(Dd // P, P, N).transpose(1, 0, 2))


def prep_inputs(inputs):
    """Host-side preprocessing: transpose x, scale/convert weights to fp8/bf16,
    fold LN gains/biases, precompute correction rows."""
    f = np.float32
    x = np.asarray(inputs["x"], f)
    lcc = np.asarray(inputs["lcc_values"], f)
    w_qkv = np.asarray(inputs["w_qkv"], f)
    b_qkv = np.asarray(inputs["b_qkv"], f)
    ln1_g = np.asarray(inputs["ln1_g"], f)
    ln1_b = np.asarray(inputs["ln1_b"], f)
    ln2_g = np.asarray(inputs["ln2_g"], f)
    ln2_b = np.asarray(inputs["ln2_b"], f)
    w_ff1 = np.asarray(inputs["w_ff1"], f)
    b_ff1 = np.asarray(inputs["b_ff1"], f)

    def chunked(b):  # [D] -> [128, DC] with chunk c in column c
        return np.ascontiguousarray(b.reshape(-1, P).T)

    wq = ln1_g[:, None] * w_qkv[:, 0:D]
    wk = ln1_g[:, None] * w_qkv[:, D:2 * D]
    wv = ln1_g[:, None] * w_qkv[:, 2 * D:3 * D]
    bq = b_qkv[0:D] + ln1_b @ w_qkv[:, 0:D]
    bk = b_qkv[D:2 * D] + ln1_b @ w_qkv[:, D:2 * D]
    bv = b_qkv[2 * D:3 * D] + ln1_b @ w_qkv[:, 2 * D:3 * D]
    wo = np.asarray(inputs["w_out"], f)
    wf1 = ln2_g[:, None] * w_ff1
    bf1f = b_ff1 + ln2_b @ w_ff1
    wf2 = np.asarray(inputs["w_ff2"], f)

    def cor_rows(w, b):
        # correction DR row: tile0 = -colsum(w)*(PS/SMU) paired with mu8,
        #                    tile1 = b*(PS/SSD) paired with sd8
        r = np.zeros((1, 2, D), f)
        r[0, 0] = -w.sum(axis=0) * (PS / SMU)
        r[0, 1] = b * (PS / SSD)
        return r.astype(NP_F8)

    xt = np.ascontiguousarray(x.T)

    # FFN weights pre-tiled for contiguous DMA: [128, FC, DC, 128]
    wf1t = np.ascontiguousarray(
        wf1.reshape(DC, P, FC, P).transpose(1, 2, 0, 3)).astype(NP_BF16)
    wf2t = np.ascontiguousarray(
        wf2.reshape(FC, P, DC, P).transpose(1, 0, 2, 3)).astype(NP_BF16)

    shared = {
        "xt": xt.astype(NP_BF16),
        "wq8": _chunk_pd(wq * SW).astype(NP_F8),
        "wk8": _chunk_pd(wk * SW).astype(NP_F8),
        "wv8": _chunk_pd(wv * SW).astype(NP_F8),
        "wo8": _chunk_pd(wo * SW).astype(NP_F8),
        "corq": cor_rows(wq, bq),
        "cork": cor_rows(wk, bk),
        "corv": cor_rows(wv, bv),
        "wf1h": wf1t,
        "wf2h": wf2t,
        "bo": chunked(np.asarray(inputs["b_out"], f)),
        "bf1": chunked(bf1f),
        "bf2": chunked(np.asarray(inputs["b_ff2"], f)),
        "lccel": np.ascontiguousarray(
            np.exp((lcc * (0.5 * LCC)).reshape(KC, P).T)),
        "selr": _selr_matrix().astype(NP_F8),
        "selrb": _selr_matrix().astype(NP_BF16),
        "selb": _selb_matrix(),
        "onesbd": np.ones((P, 1), NP_BF16),
        "ones1r": np.ones((1, P), np.float32),
        "c64r": np.full((1, P), SA * SSD / SKV, np.float32),
        "onescl": np.ones((P, 1), np.float32),
    }
    in_maps = []
    for c in range(NCORES):
        m = dict(shared)
        m["xot"] = np.ascontiguousarray(xt[:, c * LQ:(c + 1) * LQ])
        m["xotb"] = m["xot"].astype(NP_BF16)
        in_maps.append(m)
    return in_maps


def kernel(**inputs):
    nc = build()
    in_maps = prep_inputs(inputs)
    res = run_bass_kernel_spmd(nc, in_maps, core_ids=list(range(NCORES)))
    out = np.concatenate([res.results[c]["out_t"] for c in range(NCORES)], axis=1)
    return np.ascontiguousarray(out.T).astype(np.float32)


# revision 32
# speedup vs baseline: 1.6459x; 1.2973x over previous
"""Trainium2 Bass kernel for EnhancedMultiHeadSelfAttention (dense transformer block).

Sharding: sequence-parallel over 8 cores. Each core owns L/8 = 256 query rows.
K/V for all 2048 tokens are computed on every core from fp8 x; everything else
(Q, scores, softmax, attn@V, out-proj, LN2, FFN) is own-rows only. No
collectives.

Key structure (all activations feature-major [feature, token]):
 - LN1 is never applied to x. Projections run on raw fp8 x with an extra
   K=1 DoubleRow "correction row": out += (-colsum(W))*mu_t + b*sigma_t.
   The per-token rstd then cancels in cosine normalization for Q/K; for V,
   both rstd and the (key-side) lcc softmax bias are folded into a per-key
   scale applied during V's PSUM->SBUF copy, with the softmax denominator
   coming from an extra V column holding sigma*rstd*e^lcc = e^lcc terms.
 - K is cosine-normalized in place (fp8) so the exp over scores has a
   CONSTANT scale and no bias: one Activation op covers both heads of a
   pair (512 elements), halving Act-engine overhead on the exp path.
 - Q/K/V/out-proj and attn@V run as fp8e4 DoubleRow matmuls (two 128-row
   K-tiles per instruction at 0.5 cycles/row). FFN runs in bf16 (fp8 would
   exceed the 2e-2 error budget through the 4096-wide contraction).
 - clip(scores,-10,10) never binds and softmax needs no max-subtraction.

Scale conventions (stored value = scale * true value):
  x8 = 16 x     w{q,k,v,o}8 = 64 w    proj PSUM = 1024 * true
  k8 = 4 k~ then (after normalize) 16 k-hat      q8 = 16 q-hat
  v8 = 4 e^b v~ (b = key lcc bias + ln rstd)     mu8 = 256 mu
  sd8 = 16 sigma    den col = 16 e^b sigma       attn8 = 16 attn
"""

import numpy as np
import ml_dtypes

import concourse.bass as bass
import concourse.tile as tile
from concourse import bacc, mybir
from concourse.bass_utils import run_bass_kernel_spmd

F32 = mybir.dt.float32
F32R = mybir.dt.float32r
BF16 = mybir.dt.bfloat16
F8 = mybir.dt.float8e4
NP_F8 = ml_dtypes.float8_e4m3
NP_BF16 = ml_dtypes.bfloat16

L = 2048          # sequence length
D = 1024          # model dim
H = 16            # heads
DH = 64           # head dim
FF = 4096         # ffn hidden
P = 128           # partitions
NCORES = 8
LQ = L // NCORES  # 256 own query rows per core
DC = D // P       # 8 d-model chunks
FC = FF // P      # 32 ffn chunks
KC = L // P       # 16 key chunks
NBLK = 4          # token blocks of 512
BLK = L // NBLK   # 512

# CoreSim doesn't implement Gelu; test_sim swaps this to Identity and checks
# against a gelu-less reference. Hardware always uses the real (erf) Gelu.
GELU_FUNC = mybir.ActivationFunctionType.Gelu

LN_EPS = 1e-5
SCALING = DH ** -0.5
LCC = 0.1
DR = mybir.MatmulPerfMode.DoubleRow

SX = 16.0     # x8 scale
SW = 64.0     # fp8 weight scale
PS = SX * SW  # = 1024, scale of projection PSUM results
SKV = 4.0     # k~/v~ scale
SK8 = 16.0    # normalized k-hat scale
SQ8 = 16.0    # q8 scale (unit vectors * 16)
SMU = 256.0   # mu8 scale
SSD = 16.0    # sd8 (sigma) scale
SA = 16.0     # attn8 scale


def emit(tc):
    nc = tc.nc
    AF = mybir.ActivationFunctionType

    xt = nc.dram_tensor("xt", [D, L], BF16, kind="ExternalInput").ap()
    xot = nc.dram_tensor("xot", [D, LQ], F32, kind="ExternalInput").ap()
    xotb = nc.dram_tensor("xotb", [D, LQ], BF16, kind="ExternalInput").ap()
    wq8 = nc.dram_tensor("wq8", [P, DC, D], F8, kind="ExternalInput").ap()
    wk8 = nc.dram_tensor("wk8", [P, DC, D], F8, kind="ExternalInput").ap()
    wv8 = nc.dram_tensor("wv8", [P, DC, D], F8, kind="ExternalInput").ap()
    wo8 = nc.dram_tensor("wo8", [P, DC, D], F8, kind="ExternalInput").ap()
    corq = nc.dram_tensor("corq", [1, 2, D], F8, kind="ExternalInput").ap()
    cork = nc.dram_tensor("cork", [1, 2, D], F8, kind="ExternalInput").ap()
    corv = nc.dram_tensor("corv", [1, 2, D], F8, kind="ExternalInput").ap()
    wf1h = nc.dram_tensor("wf1h", [P, FC, DC, P], BF16, kind="ExternalInput").ap()
    wf2h = nc.dram_tensor("wf2h", [P, FC, DC, P], BF16, kind="ExternalInput").ap()
    bo = nc.dram_tensor("bo", [P, DC], F32, kind="ExternalInput").ap()
    bf1 = nc.dram_tensor("bf1", [P, FC], F32, kind="ExternalInput").ap()
    bf2 = nc.dram_tensor("bf2", [P, DC], F32, kind="ExternalInput").ap()
    lccel = nc.dram_tensor("lccel", [P, KC], F32, kind="ExternalInput").ap()
    selr = nc.dram_tensor("selr", [P, P], F8, kind="ExternalInput").ap()
    selrb = nc.dram_tensor("selrb", [P, P], BF16, kind="ExternalInput").ap()
    selb = nc.dram_tensor("selb", [H, DC * P], F32R, kind="ExternalInput").ap()
    onesbd = nc.dram_tensor("onesbd", [P, 1], BF16, kind="ExternalInput").ap()
    ones1r = nc.dram_tensor("ones1r", [1, P], F32R, kind="ExternalInput").ap()
    c64r = nc.dram_tensor("c64r", [1, P], F32R, kind="ExternalInput").ap()
    onescl = nc.dram_tensor("onescl", [P, 1], F32R, kind="ExternalInput").ap()
    out_t = nc.dram_tensor("out_t", [D, LQ], F32, kind="ExternalOutput").ap()

    xt3 = xt.rearrange("(c p) t -> p c t", p=P)        # [128, 8, 2048]
    xot3 = xot.rearrange("(c p) t -> p c t", p=P)      # [128, 8, 256]
    out3 = out_t.rearrange("(c p) t -> p c t", p=P)    # [128, 8, 256]

    mm = nc.tensor.matmul

    # ---- persistent small constants -------------------------------------
    singles = tc.alloc_tile_pool(name="singles", bufs=1)
    ones_1x128 = singles.tile([1, P], F32R)  # K=1 broadcast lhsT (value 1)
    nc.scalar.dma_start(ones_1x128, ones1r)
    c64row = singles.tile([1, P], F32R)      # K=1 broadcast lhsT (value 64)
    nc.scalar.dma_start(c64row, c64r)
    ones_col = singles.tile([P, 1], F32R)    # K=128 -> M=1 reduction lhsT
    nc.scalar.dma_start(ones_col, onescl)
    onesb = singles.tile([P, 1], BF16)       # bf16 reduction lhsT
    nc.scalar.dma_start(onesb, onesbd)
    selr8 = singles.tile([P, DC, H], F8)
    nc.scalar.dma_start(selr8, selr.rearrange("p (m h) -> p m h", h=H))
    selrb_sb = singles.tile([P, DC, H], BF16)
    nc.scalar.dma_start(selrb_sb, selrb.rearrange("p (m h) -> p m h", h=H))
    selb_sb = singles.tile([H, DC, P], F32R)
    nc.scalar.dma_start(selb_sb, selb.rearrange("h (m p) -> h m p", p=P))
    corq_sb = singles.tile([1, 2, D], F8)
    nc.scalar.dma_start(corq_sb, corq)
    cork_sb = singles.tile([1, 2, D], F8)
    nc.scalar.dma_start(cork_sb, cork)
    corv_sb = singles.tile([1, 2, D], F8)
    nc.scalar.dma_start(corv_sb, corv)
    bo_sb = singles.tile([P, DC], F32)
    nc.scalar.dma_start(bo_sb, bo)
    bf1_sb = singles.tile([P, FC], F32)
    nc.scalar.dma_start(bf1_sb, bf1)
    bf2_sb = singles.tile([P, DC], F32)
    nc.scalar.dma_start(bf2_sb, bf2)
    lcce_sb = singles.tile([P, KC], F32)
    nc.scalar.dma_start(lcce_sb, lccel)
    eps_sb = singles.tile([P, 1], F32)
    nc.vector.memset(eps_sb, LN_EPS)
    # fp8 full weights (4 x 8KB/partition)
    wq_sb = singles.tile([P, DC, D], F8)
    nc.sync.dma_start(wq_sb, wq8)
    wk_sb = singles.tile([P, DC, D], F8)
    nc.sync.dma_start(wk_sb, wk8)
    wv_sb = singles.tile([P, DC, D], F8)
    nc.gpsimd.dma_start(wv_sb, wv8)
    wo_sb = singles.tile([P, DC, D], F8)
    nc.gpsimd.dma_start(wo_sb, wo8)
    # per-token stat tensors (filled during phases A/B)
    musd8 = singles.tile([1, 2, L], F8)       # [mu8 ; sd8] rows
    vscale = singles.tile([P, KC], F32)       # (SKV/PS) * rstd * e^lcc
    col8 = singles.tile([P, KC], F8)          # SSD * rstd * e^lcc (den col)
    reck = singles.tile([H, L], F32R)         # 4 / |k~| rows
    stat_r = singles.tile([P, NBLK, 2, 4], F32)   # [p, b, (sum,sumsq), kc%4]
    musd_r = singles.tile([P, NBLK, 2, 4], F8)    # repartitioned mu8/sd8

    scr_pool = tc.alloc_tile_pool(name="scr", bufs=1, space="DRAM")
    scr_st = scr_pool.tile([NBLK, 2, BLK], F32)    # stats rows -> repart
    scr_ms = scr_pool.tile([NBLK, 2, BLK], F8)     # mu/sd repart -> rows

    # ---- persistent activation tensors ----------------------------------
    outp = tc.alloc_tile_pool(name="outp", bufs=1)
    x2 = outp.tile([P, DC, LQ], F32R)
    attn8 = outp.tile([P, DC, LQ], F8)
    wf1s = tc.alloc_tile_pool(name="wf1s", bufs=9)
    wf2s = tc.alloc_tile_pool(name="wf2s", bufs=6)
    midp = tc.alloc_tile_pool(name="midp", bufs=1)
    k8 = midp.tile([P, DC, L], F8)
    VW = 80  # 64 values + 1 denominator + 15 pad (dual-fp8 M%16==0)
    v_sb = midp.tile([P, KC, H, VW], F8)
    q8 = midp.tile([P, DC, LQ], F8)
    x8p = tc.alloc_tile_pool(name="x8p", bufs=1)
    x8 = x8p.tile([P, DC, L], F8)

    # =====================================================================
    # Phase A: x -> fp8, per-token stats; Phase C: Q projection + normalize
    # (emitted together so the scheduler can overlap them)
    # =====================================================================
    with (
        tc.tile_pool(name="xblk", bufs=2) as xblk_pool,
        tc.tile_pool(name="xsqp", bufs=2) as xsq_pool,
        tc.tile_pool(name="stm", bufs=2) as stm_pool,
        tc.tile_pool(name="strow", bufs=2) as strow_pool,
        tc.tile_pool(name="qcp", bufs=1) as qc_pool,
        tc.tile_pool(name="qsqp", bufs=2) as qsq_pool,
        tc.tile_pool(name="ps_st", bufs=1, space="PSUM") as ps_st,
        tc.tile_pool(name="ps_q", bufs=2, space="PSUM") as ps_q,
        tc.tile_pool(name="ps_qn", bufs=1, space="PSUM") as ps_qn,
        tc.tile_pool(name="ps_qb", bufs=1, space="PSUM") as ps_qb,
    ):
        for b in range(NBLK):
            sl = slice(b * BLK, (b + 1) * BLK)
            xblk = xblk_pool.tile([P, DC, BLK], BF16, tag="xblk")
            nc.gpsimd.dma_start(xblk, xt3[:, :, sl])
            with nc.allow_low_precision(reason="fp8 pipeline"):
                nc.scalar.activation(x8[:, :, sl], xblk, func=AF.Copy,
                                     bias=0.0, scale=SX)
                xsqb = xsq_pool.tile([P, DC, BLK], BF16, tag="xsq")
                nc.vector.tensor_mul(xsqb, xblk, xblk)
            sums = ps_st.tile([1, BLK], F32, tag="sums")
            sumsq = ps_st.tile([1, BLK], F32, tag="sumsq")
            for c in range(DC):
                mm(sums, onesb, xblk[:, c, :], start=(c == 0),
                   stop=(c == DC - 1))
                mm(sumsq, onesb, xsqb[:, c, :], start=(c == 0),
                   stop=(c == DC - 1))
            # stage stat rows to SBUF, roundtrip via DRAM to [128, ...] layout
            statrow = strow_pool.tile([1, 2, BLK], F32, tag="strow")
            nc.scalar.copy(statrow[:, 0, :], sums)
            nc.scalar.copy(statrow[:, 1, :], sumsq)
            nc.sync.dma_start(scr_st[b:b + 1], statrow)
            nc.sync.dma_start(
                stat_r[:, b, :, :],
                scr_st[b].rearrange("j (q p) -> p j q", p=P))
            # per-token coefficient math in [128, 4] layout
            mu = stm_pool.tile([P, 4], F32, tag="mu")
            nc.vector.tensor_scalar_mul(mu, stat_r[:, b, 0, :], 1.0 / D)
            ex2 = stm_pool.tile([P, 4], F32, tag="ex2")
            nc.vector.tensor_scalar_mul(ex2, stat_r[:, b, 1, :], 1.0 / D)
            var = stm_pool.tile([P, 4], F32, tag="var")
            nc.vector.tensor_mul(var, mu, mu)
            nc.vector.tensor_sub(var, ex2, var)
            sd = stm_pool.tile([P, 4], F32, tag="sd")
            nc.scalar.activation(sd, var, func=AF.Sqrt, bias=eps_sb, scale=1.0)
            rstd = stm_pool.tile([P, 4], F32, tag="rstd")
            with nc.allow_low_precision(reason="coef"):
                nc.vector.reciprocal(rstd, sd)
            kcs = slice(b * 4, (b + 1) * 4)
            relcc = stm_pool.tile([P, 4], F32, tag="relcc")
            nc.vector.tensor_mul(relcc, rstd, lcce_sb[:, kcs])
            nc.vector.tensor_scalar_mul(vscale[:, kcs], relcc, SKV / PS)
            with nc.allow_low_precision(reason="fp8 pipeline"):
                nc.vector.tensor_scalar_mul(col8[:, kcs], relcc, SSD)
                nc.vector.tensor_scalar_mul(musd_r[:, b, 0, :], mu, SMU)
                nc.vector.tensor_scalar_mul(musd_r[:, b, 1, :], sd, SSD)
            nc.sync.dma_start(
                scr_ms[b].rearrange("j (q p) -> p j q", p=P),
                musd_r[:, b, :, :])
            for j in range(2):
                nc.sync.dma_start(
                    musd8[:, j, sl],
                    scr_ms[b, j].rearrange("(o t) -> o t", o=1))

        # ---- Phase C: Q (own tokens; stats recomputed from xot since the
        # shared program can't address its own slice of musd8) --------------
        xo_blk = qc_pool.tile([P, DC, LQ], BF16, name="xo_blk")
        nc.sync.dma_start(xo_blk, xotb.rearrange("(c p) t -> p c t", p=P))
        x8own = qc_pool.tile([P, DC, LQ], F8, name="x8own")
        with nc.allow_low_precision(reason="fp8 pipeline"):
            nc.scalar.activation(x8own, xo_blk, func=AF.Copy, bias=0.0,
                                 scale=SX)
        ps_os = ps_qn.tile([1, LQ], F32, tag="osum")
        for c in range(DC):
            mm(ps_os, onesb, xo_blk[:, c, :], start=(c == 0),
               stop=(c == DC - 1))
        osr = qc_pool.tile([1, 2, LQ], F32, name="osr")
        nc.vector.tensor_copy(osr[:, 0, :], ps_os)
        xsq_o = qc_pool.tile([P, DC, LQ], BF16, name="xsq_o")
        with nc.allow_low_precision(reason="bf16 pipeline"):
            nc.vector.tensor_mul(xsq_o, xo_blk, xo_blk)
        for c in range(DC):
            mm(ps_os, onesb, xsq_o[:, c, :], start=(c == 0),
               stop=(c == DC - 1))
        nc.vector.tensor_copy(osr[:, 1, :], ps_os)
        mu_o = qc_pool.tile([1, LQ], F32, name="mu_o")
        nc.vector.tensor_scalar_mul(mu_o, osr[:, 0, :], 1.0 / D)
        ex2_o = qc_pool.tile([1, LQ], F32, name="ex2_o")
        nc.vector.tensor_scalar_mul(ex2_o, osr[:, 1, :], 1.0 / D)
        var_o = qc_pool.tile([1, LQ], F32, name="var_o")
        nc.vector.tensor_mul(var_o, mu_o, mu_o)
        nc.vector.tensor_sub(var_o, ex2_o, var_o)
        sd_o = qc_pool.tile([1, LQ], F32, name="sd_o")
        nc.scalar.activation(sd_o, var_o, func=AF.Sqrt, bias=eps_sb[0:1, :],
                             scale=1.0)
        musd_own = qc_pool.tile([1, 2, LQ], F8, name="musd_own")
        with nc.allow_low_precision(reason="fp8 pipeline"):
            nc.vector.tensor_scalar_mul(musd_own[:, 0, :], mu_o, SMU)
            nc.vector.tensor_scalar_mul(musd_own[:, 1, :], sd_o, SSD)
        # Q DoubleRow projections + per-head cosine normalization
        qt_sb = qc_pool.tile([P, DC, LQ], BF16, name="qt_sb")
        nsq_q = ps_qn.tile([H, LQ], F32, tag="qn")
        qsq8 = None
        for m in range(DC):
            ps = ps_q.tile([P, LQ], F32, tag="qps")
            for i in range(4):
                mm(ps, wq_sb[:, 2 * i:2 * i + 2, m * P:(m + 1) * P],
                   x8own[:, 2 * i:2 * i + 2, :], start=(i == 0), stop=False,
                   perf_mode=DR)
            mm(ps, corq_sb[:, :, m * P:(m + 1) * P], musd_own,
               start=False, stop=True, perf_mode=DR)
            with nc.allow_low_precision(reason="bf16 pipeline"):
                nc.vector.tensor_scalar_mul(qt_sb[:, m, :], ps, SKV / PS)
            if m % 2 == 0:
                qsq8 = qsq_pool.tile([P, 2, LQ], F8, tag="qsq")
            nc.scalar.activation(qsq8[:, m % 2, :], ps, func=AF.Square,
                                 bias=0.0, scale=2.0 / PS)
            if m % 2 == 1:
                mm(nsq_q, selr8[:, m - 1:m + 1, :], qsq8,
                   start=(m == 1), stop=(m == DC - 1), perf_mode=DR)
        qs = qc_pool.tile([H, LQ], F32, name="qs")
        nc.scalar.activation(qs, nsq_q, func=AF.Sqrt, bias=0.0, scale=1.0)
        nc.vector.tensor_scalar_max(qs, qs, 1e-8)
        rec = qc_pool.tile([H, LQ], F32R, name="qrec")
        with nc.allow_low_precision(reason="coef"):
            nc.vector.reciprocal(rec, qs)
        nc.vector.tensor_scalar_mul(rec, rec, 2.0 * SQ8 / SKV)
        for m in range(DC):
            bc = ps_qb.tile([P, LQ], F32, tag="qbc")
            mm(bc, selb_sb[:, m, :], rec, start=True, stop=True)
            with nc.allow_low_precision(reason="fp8 pipeline"):
                nc.vector.tensor_mul(q8[:, m, :], qt_sb[:, m, :], bc)

    # sigma/e^lcc column of V (denominator source) + zero pad columns
    with nc.allow_low_precision(reason="fp8 pipeline"):
        nc.vector.memset(v_sb[:, :, :, DH + 1:VW], 0.0)
        nc.vector.tensor_copy(
            v_sb[:, :, :, DH], col8.unsqueeze(2).to_broadcast([P, KC, H]))

    # =====================================================================
    # Phase B: K and V projections (fp8 DR); K cosine-normalized in place
    # =====================================================================
    with (
        tc.tile_pool(name="ksqp", bufs=2) as ksq_pool,
        tc.tile_pool(name="nsqs", bufs=2) as nsqs_pool,
        tc.tile_pool(name="ps_k", bufs=2, space="PSUM") as ps_k,
        tc.tile_pool(name="ps_v", bufs=2, space="PSUM") as ps_v,
        tc.tile_pool(name="ps_n", bufs=2, space="PSUM") as ps_n,
    ):
        for b in range(NBLK):
            sl = slice(b * BLK, (b + 1) * BLK)
            ms = musd8[:, :, sl]
            for m in range(DC):
                ps = ps_k.tile([P, BLK], F32, tag="kps")
                for i in range(4):
                    mm(ps, wk_sb[:, 2 * i:2 * i + 2, m * P:(m + 1) * P],
                       x8[:, 2 * i:2 * i + 2, sl], start=(i == 0), stop=False,
                       perf_mode=DR)
                mm(ps, cork_sb[:, :, m * P:(m + 1) * P], ms,
                   start=False, stop=True, perf_mode=DR)
                with nc.allow_low_precision(reason="fp8 pipeline"):
                    if m % 2 == 0:
                        nc.vector.tensor_scalar_mul(k8[:, m, sl], ps, SKV / PS)
                    else:
                        nc.scalar.activation(k8[:, m, sl], ps, func=AF.Copy,
                                             bias=0.0, scale=SKV / PS)
            # V for this block's 4 token chunks (scaled per key by vscale)
            for t in range(b * 4, (b + 1) * 4):
                tsl = slice(t * P, (t + 1) * P)
                for g in range(2):
                    csl = slice(g * BLK, (g + 1) * BLK)
                    ps = ps_v.tile([P, BLK], F32, tag="vps")
                    for i in range(4):
                        mm(ps, x8[:, 2 * i:2 * i + 2, tsl],
                           wv_sb[:, 2 * i:2 * i + 2, csl],
                           start=(i == 0), stop=False, perf_mode=DR)
                    mm(ps, musd8[:, :, tsl], corv_sb[:, :, csl],
                       start=False, stop=True, perf_mode=DR)
                    ps_h = ps.rearrange("p (h d) -> p h d", d=DH)
                    dst = v_sb[:, t, g * DC:(g + 1) * DC, 0:DH]
                    with nc.allow_low_precision(reason="fp8 pipeline"):
                        if g == 0:
                            nc.vector.tensor_scalar_mul(
                                dst, ps_h, vscale[:, t:t + 1])
                        else:
                            nc.scalar.activation(dst, ps_h, func=AF.Copy,
                                                 bias=0.0,
                                                 scale=vscale[:, t:t + 1])
            # k norms: squares on Pool (bf16), per-head sums, then reck
            nsq = ps_n.tile([H, BLK], F32, tag="nsq")
            for m in range(DC):
                ksqb = ksq_pool.tile([P, BLK], BF16, tag="ksq")
                with nc.allow_low_precision(reason="bf16 pipeline"):
                    nc.gpsimd.tensor_mul(ksqb, k8[:, m, sl], k8[:, m, sl])
                mm(nsq, selrb_sb[:, m, :], ksqb, start=(m == 0),
                   stop=(m == DC - 1))
            nsq_sb = nsqs_pool.tile([H, BLK], F32, tag="nsqs")
            nc.scalar.activation(nsq_sb, nsq, func=AF.Sqrt, bias=0.0,
                                 scale=1.0)
            nc.vector.tensor_scalar_max(nsq_sb, nsq_sb, 1e-8)
            with nc.allow_low_precision(reason="coef"):
                nc.vector.reciprocal(reck[:, sl], nsq_sb)
            nc.vector.tensor_scalar_mul(reck[:, sl], reck[:, sl], SK8)

    x8p.release()

    # =====================================================================
    # Phases D/E/F (full query width): scores -> exp -> attn@V -> out-proj
    # -> LN2 -> FFN
    # =====================================================================
    EXPS = SCALING / (SQ8 * SK8)

    ffp = tc.alloc_tile_pool(name="ffp", bufs=1)
    h_t = ffp.tile([P, FC, LQ], BF16)
    normed2 = ffp.tile([P, DC, LQ], BF16)
    dep = tc.alloc_tile_pool(name="dep", bufs=1)
    xo2 = dep.tile([P, DC, LQ], F32)
    nc.sync.dma_start(xo2, xot3)

    with (
        tc.tile_pool(name="ehp", bufs=3) as eh_pool,
        tc.tile_pool(name="rcp", bufs=2) as rc_pool,
        tc.tile_pool(name="ps_sc", bufs=2, space="PSUM") as ps_sc,
        tc.tile_pool(name="ps_ac", bufs=1, space="PSUM") as ps_ac,
        tc.tile_pool(name="ps_rb", bufs=1, space="PSUM") as ps_rb,
        tc.tile_pool(name="ps_kb", bufs=2, space="PSUM") as ps_kb,
    ):
        for m in range(DC):
            # normalize this head-pair's K in place (k8 -> 16 * k-hat);
            # overlaps the previous pair's Act-bound exp work
            for b in range(NBLK):
                sl = slice(b * BLK, (b + 1) * BLK)
                kb = ps_kb.tile([P, BLK], F32, tag="kbc", name="kbps")
                mm(kb, selb_sb[:, m, :], reck[:, sl], start=True, stop=True)
                with nc.allow_low_precision(reason="fp8 pipeline"):
                    nc.vector.tensor_mul(k8[:, m, sl], k8[:, m, sl], kb)
            eh8 = eh_pool.tile([P, KC, 2, LQ], F8, tag="eh", name="eh8")
            for kc in range(KC):
                ps = ps_sc.tile([P, 2, 2 * LQ], F32, tag="sc", name="scps")
                for j in range(2):
                    mm(ps[:, j, 0:LQ],
                       k8[j * DH:(j + 1) * DH, m, kc * P:(kc + 1) * P],
                       q8[j * DH:(j + 1) * DH, m, :], start=True, stop=True)
                with nc.allow_low_precision(reason="fp8 pipeline"):
                    nc.scalar.activation(eh8[:, kc, :, :], ps[:, :, 0:LQ],
                                         func=AF.Exp, bias=0.0, scale=EXPS)
            for j in range(2):
                acc = ps_ac.tile([VW, LQ], F32, tag="ac", name="accps")
                for i in range(KC // 2):
                    mm(acc, v_sb[:, 2 * i:2 * i + 2, 2 * m + j, :],
                       eh8[:, 2 * i:2 * i + 2, j, :],
                       start=(i == 0), stop=(i == KC // 2 - 1), perf_mode=DR)
                recip = rc_pool.tile([1, LQ], F32R, tag="recip", name="recip")
                with nc.allow_low_precision(reason="coef"):
                    nc.vector.reciprocal(recip, acc[DH:DH + 1, :])
                rbc = ps_rb.tile([DH, LQ], F32, tag="rb", name="rbcps")
                mm(rbc, c64row[:, 0:DH], recip, start=True, stop=True)
                rbc_sb = rc_pool.tile([DH, LQ], F32, tag="rbcsb",
                                      name="rbcsb")
                nc.vector.tensor_copy(rbc_sb, rbc)
                with nc.allow_low_precision(reason="fp8 pipeline"):
                    nc.vector.tensor_mul(
                        attn8[j * DH:(j + 1) * DH, m, :], acc[0:DH, :],
                        rbc_sb)

    # out-projection + residual -> x2; LN2 -> normed2
    with (
        tc.tile_pool(name="upp", bufs=2) as up_pool,
        tc.tile_pool(name="lnc0", bufs=1) as lnc0,
        tc.tile_pool(name="ps_eo", bufs=2, space="PSUM") as ps_eo,
        tc.tile_pool(name="ps_l", bufs=1, space="PSUM") as ps_l,
        tc.tile_pool(name="ps_lb", bufs=1, space="PSUM") as ps_lb,
    ):
        for o in range(DC):
            ps = ps_eo.tile([P, LQ], F32, tag="ops", name="ops")
            for i in range(4):
                mm(ps, wo_sb[:, 2 * i:2 * i + 2, o * P:(o + 1) * P],
                   attn8[:, 2 * i:2 * i + 2, :], start=(i == 0),
                   stop=(i == 3), perf_mode=DR)
            upd = up_pool.tile([P, LQ], F32, tag="upd", name="upd")
            nc.scalar.activation(upd, ps, func=AF.Identity,
                                 bias=bo_sb[:, o:o + 1], scale=1.0 / (SA * SW))
            with nc.allow_low_precision(reason="f32r"):
                nc.vector.tensor_add(x2[:, o, :], upd, xo2[:, o, :])
        sums = ps_l.tile([1, LQ], F32, tag="lsum", name="lsums")
        sumsq = ps_l.tile([1, LQ], F32, tag="lsumsq", name="lsumsq")
        for c in range(DC):
            xsq = lnc0.tile([P, LQ], F32R, tag="lxsq", name="lxsq", bufs=2)
            nc.scalar.square(xsq, x2[:, c, :])
            mm(sums, ones_col, x2[:, c, :], start=(c == 0),
               stop=(c == DC - 1))
            mm(sumsq, ones_col, xsq, start=(c == 0), stop=(c == DC - 1))
        mu = lnc0.tile([1, LQ], F32, tag="lmu", name="lmu")
        nc.vector.tensor_scalar_mul(mu, sums, 1.0 / D)
        ex2 = lnc0.tile([1, LQ], F32, tag="lex2", name="lex2")
        nc.vector.tensor_scalar_mul(ex2, sumsq, 1.0 / D)
        var = lnc0.tile([1, LQ], F32, tag="lvar", name="lvar")
        nc.vector.tensor_mul(var, mu, mu)
        nc.vector.tensor_sub(var, ex2, var)
        sd = lnc0.tile([1, LQ], F32, tag="lsd", name="lsd")
        nc.scalar.activation(sd, var, func=AF.Sqrt, bias=eps_sb[0:1, :],
                             scale=1.0)
        coef = lnc0.tile([1, 2, LQ], F32R, tag="lcoef2", name="lcoef2")
        with nc.allow_low_precision(reason="coef"):
            nc.vector.reciprocal(coef[:, 0, :], sd)
            nc.vector.tensor_mul(coef[:, 1, :], mu, coef[:, 0, :])
            nc.vector.tensor_scalar_mul(coef[:, 1, :], coef[:, 1, :], -1.0)
        bc = ps_lb.tile([P, 2, LQ], F32, tag="lbc", name="lbc")
        mm(bc, ones_1x128, coef, start=True, stop=True)
        shift_sb = lnc0.tile([P, LQ], F32, tag="lshsb", name="lshsb")
        nc.scalar.copy(shift_sb, bc[:, 1, :])
        rb = bc[:, 0, :].unsqueeze(1).to_broadcast([P, DC, LQ])
        sb = shift_sb.unsqueeze(1).to_broadcast([P, DC, LQ])
        with nc.allow_low_precision(reason="bf16 pipeline"):
            nc.vector.tensor_mul(normed2, x2, rb)
            nc.gpsimd.tensor_add(normed2, normed2, sb)

    # FFN
    with (
        tc.tile_pool(name="osbp", bufs=2) as osb_pool,
        tc.tile_pool(name="ps_f1", bufs=3, space="PSUM") as ps_f1,
        tc.tile_pool(name="ps_f2", bufs=4, space="PSUM") as ps_f2,
    ):
        for f in range(FC):
            wf1m = wf1s.tile([P, DC, P], BF16, tag="wf1", name="wf1m")
            weng = nc.sync if f % 2 == 0 else nc.gpsimd
            weng.dma_start(wf1m, wf1h[:, f, :, :])
            ps = ps_f1.tile([P, LQ], F32, tag="f1", name="f1ps")
            for c in range(DC):
                mm(ps, wf1m[:, c, :], normed2[:, c, :], start=(c == 0),
                   stop=(c == DC - 1))
            with nc.allow_low_precision(reason="bf16 pipeline"):
                nc.scalar.activation(h_t[:, f, :], ps, func=GELU_FUNC,
                                     bias=bf1_sb[:, f:f + 1], scale=1.0)
        for g in range(2):
            accs = [ps_f2.tile([P, LQ], F32, tag="f2acc",
                               name=f"f2acc{i}") for i in range(4)]
            for f in range(FC):
                wf2m = wf2s.tile([P, 4, P], BF16, tag="wf2", name="wf2m")
                weng2 = nc.gpsimd if f % 2 == 0 else nc.sync
                weng2.dma_start(wf2m, wf2h[:, f, g * 4:(g + 1) * 4, :])
                for i in range(4):
                    mm(accs[i], wf2m[:, i, :], h_t[:, f, :],
                       start=(f == 0), stop=(f == FC - 1))
            for i in range(4):
                o = g * 4 + i
                osb = osb_pool.tile([P, LQ], F32, tag="osb", name="osb")
                nc.scalar.activation(osb, accs[i], func=AF.Identity,
                                     bias=bf2_sb[:, o:o + 1], scale=1.0)
                nc.vector.tensor_add(osb, osb, x2[:, o, :])
                nc.sync.dma_start(out3[:, o, :], osb)

    dep.release()
    ffp.release()
    midp.release()
    wf2s.release()
    wf1s.release()
    outp.release()
    scr_pool.release()
    singles.release()


_CACHED = None


def build():
    global _CACHED
    if _CACHED is None:
        nc = bacc.Bacc("TRN2", target_bir_lowering=False, debug=False)
        with tile.TileContext(nc) as tc:
            emit(tc)
        nc.compile()
        _CACHED = nc
    return _CACHED


def _selr_matrix():
    # [P, DC*H]: selr[p, m*16+h] = 1 iff h == 2m + (p >= 64)
    s = np.zeros((P, DC, H), np.float32)
    for m in range(DC):
        s[0:DH, m, 2 * m] = 1.0
        s[DH:P, m, 2 * m + 1] = 1.0
    return np.ascontiguousarray(s.reshape(P, P))


def _selb_matrix():
    # [H, DC*P]: selb[h, m*128+p] = 1 iff h == 2m + (p >= 64)
    s = np.zeros((H, DC, P), np.float32)
    for m in range(DC):
        s[2 * m, m, 0:DH] = 1.0
        s[2 * m + 1, m, DH:P] = 1.0
    return np.ascontiguousarray(s.reshape(H, DC * P))


def _chunk_pd(w):
    """[D, N] -> [128, D//128, N] with (p, c, n) = w[c*128+p, n]."""
    Dd, N = w.shape
    return np.ascontiguousarray(w.reshape(Dd // P, P, N).transpose(1, 0, 2))


def prep_inputs(inputs):
    """Host-side preprocessing: transpose x, scale/convert weights to fp8/bf16,
    fold LN gains/biases, precompute correction rows."""
    f = np.float32
    x = np.asarray(inputs["x"], f)
    lcc = np.asarray(inputs["lcc_values"], f)
    w_qkv = np.asarray(inputs["w_qkv"], f)
    b_qkv = np.asarray(inputs["b_qkv"], f)
    ln1_g = np.asarray(inputs["ln1_g"], f)
    ln1_b = np.asarray(inputs["ln1_b"], f)
    ln2_g = np.asarray(inputs["ln2_g"], f)
    ln2_b = np.asarray(inputs["ln2_b"], f)
    w_ff1 = np.asarray(inputs["w_ff1"], f)
    b_ff1 = np.asarray(inputs["b_ff1"], f)

    def chunked(b):  # [D] -> [128, DC] with chunk c in column c
        return np.ascontiguousarray(b.reshape(-1, P).T)

    wq = ln1_g[:, None] * w_qkv[:, 0:D]
    wk = ln1_g[:, None] * w_qkv[:, D:2 * D]
    wv = ln1_g[:, None] * w_qkv[:, 2 * D:3 * D]
    bq = b_qkv[0:D] + ln1_b @ w_qkv[:, 0:D]
    bk = b_qkv[D:2 * D] + ln1_b @ w_qkv[:, D:2 * D]
    bv = b_qkv[2 * D:3 * D] + ln1_b @ w_qkv[:, 2 * D:3 * D]
    wo = np.asarray(inputs["w_out"], f)
    wf1 = ln2_g[:, None] * w_ff1
    bf1f = b_ff1 + ln2_b @ w_ff1
    wf2 = np.asarray(inputs["w_ff2"], f)

    def cor_rows(w, b):
        # correction DR row: tile0 = -colsum(w)*(PS/SMU) paired with mu8,
        #                    tile1 = b*(PS/SSD) paired with sd8
        r = np.zeros((1, 2, D), f)
        r[0, 0] = -w.sum(axis=0) * (PS / SMU)
        r[0, 1] = b * (PS / SSD)
        return r.astype(NP_F8)

    xt = np.ascontiguousarray(x.T)

    # FFN weights pre-tiled for contiguous DMA: [128, FC, DC, 128]
    wf1t = np.ascontiguousarray(
        wf1.reshape(DC, P, FC, P).transpose(1, 2, 0, 3)).astype(NP_BF16)
    wf2t = np.ascontiguousarray(
        wf2.reshape(FC, P, DC, P).transpose(1, 0, 2, 3)).astype(NP_BF16)

    shared = {
        "xt": xt.astype(NP_BF16),
        "wq8": _chunk_pd(wq * SW).astype(NP_F8),
        "wk8": _chunk_pd(wk * SW).astype(NP_F8),
        "wv8": _chunk_pd(wv * SW).astype(NP_F8),
        "wo8": _chunk_pd(wo * SW).astype(NP_F8),
        "corq": cor_rows(wq, bq),
        "cork": cor_rows(wk, bk),
        "corv": cor_rows(wv, bv),
        "wf1h": wf1t,
        "wf2h": wf2t,
        "bo": chunked(np.asarray(inputs["b_out"], f)),
        "bf1": chunked(bf1f),
        "bf2": chunked(np.asarray(inputs["b_ff2"], f)),
        "lccel": np.ascontiguousarray(
            np.exp((lcc * (0.5 * LCC)).reshape(KC, P).T)),
        "selr": _selr_matrix().astype(NP_F8),
        "selrb": _selr_matrix().astype(NP_BF16),
        "selb": _selb_matrix(),
        "onesbd": np.ones((P, 1), NP_BF16),
        "ones1r": np.ones((1, P), np.float32),
        "c64r": np.full((1, P), SA * SSD / SKV, np.float32),
        "onescl": np.ones((P, 1), np.float32),
    }
    in_maps = []
    for c in range(NCORES):
        m = dict(shared)
        m["xot"] = np.ascontiguousarray(xt[:, c * LQ:(c + 1) * LQ])
        m["xotb"] = m["xot"].astype(NP_BF16)
        in_maps.append(m)
    return in_maps


def kernel(**inputs):
    nc = build()
    in_maps = prep_inputs(inputs)
    res = run_bass_kernel_spmd(nc, in_maps, core_ids=list(range(NCORES)))
    out = np.concatenate([res.results[c]["out_t"] for c in range(NCORES)], axis=1)
    return np.ascontiguousarray(out.T).astype(np.float32)
